# revision 16
# baseline (speedup 1.0000x reference)
"""Trainium2 Bass kernel for the multimodal BERT fusion block.

Contract: kernel(**inputs) takes FULL unsharded numpy inputs (as produced by
setup_inputs()), runs an SPMD Bass kernel on 8 NeuronCores (data-parallel over
the batch dim, params replicated), and returns the FULL outputs
(h[:,0], text_att1, fusion_att1) as numpy arrays.

Math per batch b (S=512 tokens, H=768, P=30 proj dim, FD=74 audio feat):
  textT[b]  = Wt @ hidden[b]^T                    [30, 512]
  ssq       = sum(textT^2) over ALL batches       (global -> AllReduce)
  inv_w2    = ssq^-0.5   (w = ssq^0.25; text/w gram scale = 1/w^2)
  text_att1 = relu((textT^T textT) * inv_w2)      [512, 512]   (output)
  audT[b]   = Wa @ audio[b]^T                     [30, 512]
  audio_att = relu(audT^T audT)
  F_pre     = tw*text_att1 + aw*audio_att ; fusion_att1 = relu(F_pre+fb) (output)
  row0: softmax(F_pre[0,:] + am[b,:]) -> e/sum ;  fd0 = e@hidden[b]/sum + hidden[b,0]
  h0 = LN(fd0 @ dW^T + db) * ln_w + ln_b          [768]        (output h[:,0])

Implementation notes:
  - hidden^T is built on-chip via PE transposes (f32 DMA transpose does not
    exist on trn2) and kept resident in fp32r; it feeds both the text
    projection (PE, fp32r) and the softmax-weighted row-0 reduction (DVE
    tensor_tensor_reduce dot products).
  - All large matmuls run in fp32r (1 cycle/row at N>=256 vs 4 for fp32).
  - A tiny scalar AllReduce distributes the global text norm.
"""
import ctypes
import contextlib
import sys
import types

import numpy as np

import concourse.bass as bass
import concourse.bacc as bacc
import concourse.mybir as mybir
import concourse.tile as tile
from concourse import masks
from concourse.bass_utils import run_bass_kernel_spmd

B, S, H, FD, P = 64, 512, 768, 74, 30
N_CORES = 8
BPC = B // N_CORES          # 8 batches per core
HC, SC = H // 128, S // 128  # 6, 4
LN_EPS = 1e-12

F32 = mybir.dt.float32
F32R = mybir.dt.float32r
AF = mybir.ActivationFunctionType
ALU = mybir.AluOpType
AX = mybir.AxisListType

PROFILE = False          # set True (from test.py) to capture HW exec time
LAST_EXEC_NS = None

_compiled = {}


def _build(tw: float, aw: float, fb: float):
    nc = bacc.Bacc("TRN2", target_bir_lowering=False, debug=False,
                   num_devices=N_CORES)

    hid_d = nc.dram_tensor("hid", [BPC, S, H], F32, kind="ExternalInput").ap()
    aud_d = nc.dram_tensor("aud", [BPC, S, FD], F32, kind="ExternalInput").ap()
    am_d = nc.dram_tensor("am", [BPC, S], F32, kind="ExternalInput").ap()
    wt_d = nc.dram_tensor("wt", [P, H], F32, kind="ExternalInput").ap()
    wa_d = nc.dram_tensor("wa", [P, FD], F32, kind="ExternalInput").ap()
    dw_d = nc.dram_tensor("dw", [H, H], F32, kind="ExternalInput").ap()
    db_d = nc.dram_tensor("db", [H], F32, kind="ExternalInput").ap()
    lnw_d = nc.dram_tensor("lnw", [H], F32, kind="ExternalInput").ap()
    lnb_d = nc.dram_tensor("lnb", [H], F32, kind="ExternalInput").ap()

    tatt_d = nc.dram_tensor("t_att", [BPC, S, S], F32, kind="ExternalOutput").ap()
    fatt_d = nc.dram_tensor("f_att", [BPC, S, S], F32, kind="ExternalOutput").ap()
    out0_d = nc.dram_tensor("out0", [BPC, H], F32, kind="ExternalOutput").ap()

    with tile.TileContext(nc) as tc:
        _body(nc, tc, tw, aw, fb,
              hid_d, aud_d, am_d, wt_d, wa_d, dw_d, db_d, lnw_d, lnb_d,
              tatt_d, fatt_d, out0_d)
    nc.compile()
    return nc


def _body(nc, tc, tw, aw, fb,
          hid_d, aud_d, am_d, wt_d, wa_d, dw_d, db_d, lnw_d, lnb_d,
          tatt_d, fatt_d, out0_d):
    from contextlib import ExitStack
    ctx = ExitStack()
    with ctx:
        const = ctx.enter_context(tc.tile_pool(name="const", bufs=1))
        persist = ctx.enter_context(tc.tile_pool(name="persist", bufs=1))
        work = ctx.enter_context(tc.tile_pool(name="work", bufs=2))
        att = ctx.enter_context(tc.tile_pool(name="att", bufs=4))
        smalls = ctx.enter_context(tc.tile_pool(name="smalls", bufs=2))
        dram = ctx.enter_context(tc.tile_pool(name="dram", bufs=1, space="DRAM"))

        ps_tr = ctx.enter_context(tc.tile_pool(name="ps_tr", bufs=3, space="PSUM"))
        ps_mm = ctx.enter_context(tc.tile_pool(name="ps_mm", bufs=2, space="PSUM"))
        ps_g = ctx.enter_context(tc.tile_pool(name="ps_g", bufs=2, space="PSUM"))

        # ---------------- constants / params ----------------
        ident = const.tile([128, 128], F32)
        masks.make_identity(nc, ident[:])

        ones_p = const.tile([P, 1], F32)
        nc.gpsimd.memset(ones_p[:], 1.0)

        # Wt^T: [P, H] -> HC chunks [128, P]
        wt_nat = const.tile([P, H], F32)
        nc.sync.dma_start(wt_nat[:], wt_d[:])
        wtT = const.tile([128, HC, P], F32R)
        for c in range(HC):
            tp = ps_tr.tile([128, P], F32, tag="tr")
            nc.tensor.transpose(tp[:], wt_nat[:, c * 128:(c + 1) * 128],
                                ident[:P, :P])
            nc.scalar.copy(wtT[:, c, :], tp[:])

        # Wa^T: [P, FD] -> [FD, P]
        wa_nat = const.tile([P, FD], F32)
        nc.sync.dma_start(wa_nat[:], wa_d[:])
        waT = const.tile([FD, P], F32R)
        tpw = ps_tr.tile([FD, P], F32, tag="tr")
        nc.tensor.transpose(tpw[:], wa_nat[:], ident[:P, :P])
        nc.scalar.copy(waT[:], tpw[:])

        # ln/db replicated across BPC partitions via broadcast-read DMA
        db_r = const.tile([BPC, H], F32)
        lnw_r = const.tile([BPC, H], F32)
        lnb_r = const.tile([BPC, H], F32)
        for dst, src in ((db_r, db_d), (lnw_r, lnw_d), (lnb_r, lnb_d)):
            s1 = src[:].rearrange("(o h) -> o h", o=1)
            nc.sync.dma_start(dst[:], s1.to_broadcast((BPC, H)))

        # ---------------- persistent state ----------------
        hidT_all = persist.tile([128, BPC, HC, S], F32R)    # hidden^T, fp32r
        textT_all = persist.tile([P, BPC, S], F32R)         # raw text proj
        ssq_all = persist.tile([P, BPC], F32)
        fd_all = persist.tile([128, HC, BPC], F32R)

        # ---------------- phase 1: load hidden, transpose, text proj ----
        ph1_cm = tc.tile_pool(name="ph1", bufs=2)
        ph1 = ph1_cm.__enter__()
        for b in range(BPC):
            for half in range(2):
                hid_nat = ph1.tile([128, 2, H], F32, tag="hidnat", bufs=2,
                                   name=f"hn{b}_{half}")
                for i2 in range(2):
                    i = 2 * half + i2
                    nc.sync.dma_start(hid_nat[:, i2, :],
                                      hid_d[b, i * 128:(i + 1) * 128, :])
                for c in range(HC):
                    for i2 in range(2):
                        i = 2 * half + i2
                        trp = ps_tr.tile([128, 128], F32, tag="tr")
                        nc.tensor.transpose(
                            trp[:], hid_nat[:, i2, c * 128:(c + 1) * 128],
                            ident[:, :])
                        dst = hidT_all[:, b, c, i * 128:(i + 1) * 128]
                        if (c + i) % 2 == 0:
                            nc.scalar.copy(dst, trp[:])
                        else:
                            nc.vector.tensor_copy(dst, trp[:])
            tx_ps = ps_mm.tile([P, S], F32, tag="proj")
            for c in range(HC):
                nc.tensor.matmul(tx_ps[:], wtT[:, c, :], hidT_all[:, b, c, :],
                                 start=(c == 0), stop=(c == HC - 1))
            nc.scalar.copy(textT_all[:, b, :], tx_ps[:])
            sq_ps = ps_mm.tile([P, S], F32, tag="proj")
            nc.scalar.activation(sq_ps[:], tx_ps[:], AF.Square,
                                 accum_out=ssq_all[:, b:b + 1])

        ph1_cm.__exit__(None, None, None)

        # ---------------- global sum-of-squares (AllReduce) ----------------
        ssq_vec = smalls.tile([P, 1], F32, bufs=1)
        nc.vector.reduce_sum(ssq_vec[:], ssq_all[:], axis=AX.X)
        ssq_ps = ps_mm.tile([1, 1], F32, tag="proj")
        nc.tensor.matmul(ssq_ps[:], ones_p[:], ssq_vec[:])
        ssq_sc = smalls.tile([1, 1], F32, bufs=1)
        nc.scalar.copy(ssq_sc[:], ssq_ps[:])
        cc_in = dram.tile([1, 1], F32)
        cc_out = dram.tile([1, 1], F32)
        nc.gpsimd.dma_start(cc_in[:], ssq_sc[:])
        nc.gpsimd.collective_compute(
            "AllReduce", ALU.add,
            replica_groups=[list(range(N_CORES))],
            ins=[cc_in.opt()], outs=[cc_out.opt()])
        ssq_g = smalls.tile([1, 1], F32, bufs=1)
        nc.gpsimd.dma_start(ssq_g[:], cc_out[:])
        w2 = smalls.tile([1, 1], F32, bufs=1)
        nc.scalar.sqrt(w2[:], ssq_g[:])
        inv_w2 = smalls.tile([1, 1], F32, bufs=1)
        nc.vector.reciprocal(inv_w2[:], w2[:])
        inv_w2_r = smalls.tile([128, 1], F32, bufs=1)
        nc.gpsimd.partition_broadcast(inv_w2_r[:], inv_w2[:])

        # ---------------- dW^T build (fills the collective window) --------
        dwT = persist.tile([128, HC, H], F32R)
        for oc in range(HC):
            dw_nat = work.tile([128, H], F32, tag="dwnat", bufs=1)
            nc.sync.dma_start(dw_nat[:], dw_d[oc * 128:(oc + 1) * 128, :])
            for hc in range(HC):
                tp2 = ps_tr.tile([128, 128], F32, tag="tr")
                nc.tensor.transpose(tp2[:], dw_nat[:, hc * 128:(hc + 1) * 128],
                                    ident[:, :])
                if (oc + hc) % 2 == 0:
                    nc.scalar.copy(dwT[:, hc, oc * 128:(oc + 1) * 128], tp2[:])
                else:
                    nc.vector.tensor_copy(dwT[:, hc, oc * 128:(oc + 1) * 128],
                                          tp2[:])

        # ---------------- audio pipeline + gram precompute ----------------
        BF16 = mybir.dt.bfloat16
        # batches whose relu(G_t) / relu(aw*G_a) are precomputed (in bf16)
        # during the collective window; relu commutes with the positive
        # 1/w^2 scale, so relu(G_t) can be built before the AllReduce lands.
        N_PRE = 2 if (aw >= 0.0 and tw >= 0.0) else 0
        bank = {}

        def audio_proc(b):
            """load+transpose+project audio for batch b -> audT [P, S] f32r"""
            aud_ps = ps_tr.tile([FD, S], F32, tag="atr", bufs=1, name=f"aud_ps{b}")
            for i in range(SC):
                ach = work.tile([128, FD], F32, tag="ach", bufs=2, name=f"ach{b}_{i}")
                nc.sync.dma_start(ach[:], aud_d[b, i * 128:(i + 1) * 128, :])
                nc.tensor.transpose(aud_ps[:, i * 128:(i + 1) * 128],
                                    ach[:], ident[:, :])
            audT_raw = work.tile([FD, S], F32R, tag="audraw", bufs=2, name=f"ar{b}")
            if b % 2 == 0:
                nc.scalar.copy(audT_raw[:], aud_ps[:])
            else:
                nc.vector.tensor_copy(audT_raw[:], aud_ps[:])
            ap_ps = ps_mm.tile([P, S], F32, tag="proj", name=f"ap_ps{b}")
            nc.tensor.matmul(ap_ps[:], waT[:], audT_raw[:])
            audT = work.tile([P, S], F32R, tag="audT", bufs=3, name=f"audT{b}")
            nc.scalar.copy(audT[:], ap_ps[:])
            return audT

        def relu_gt_raw(b, m, dtype, pool, tag, bufs):
            """relu(G_t raw) for (b, m) -> tile (scale by inv_w2 later)"""
            idx = b * SC + m
            msl = slice(m * 128, (m + 1) * 128)
            gt_ps = ps_g.tile([128, S], F32, tag="g", name=f"gt{b}_{m}")
            nc.tensor.matmul(gt_ps[:], textT_all[:, b, msl],
                             textT_all[:, b, :])
            t = pool.tile([128, S], dtype, tag=tag, bufs=bufs, name=f"t{b}_{m}")
            if idx % 2 == 0:
                nc.scalar.activation(t[:], gt_ps[:], AF.Relu)
            else:
                nc.vector.tensor_scalar_max(t[:], gt_ps[:], 0.0)
            return t

        def relu_ga(b, m, audT, dtype, pool, tag, bufs):
            """relu(aw * G_a) for (b, m) -> tile in `pool`"""
            idx = b * SC + m
            msl = slice(m * 128, (m + 1) * 128)
            ga_ps = ps_g.tile([128, S], F32, tag="g", name=f"ga{b}_{m}")
            nc.tensor.matmul(ga_ps[:], audT[:, msl], audT[:])
            r_t = pool.tile([128, S], dtype, tag=tag, bufs=bufs,
                            name=f"r{b}_{m}")
            if aw >= 0.0:
                if aw == 1.0 and idx % 2 == 0:
                    nc.vector.tensor_scalar_max(r_t[:], ga_ps[:], 0.0)
                else:
                    nc.scalar.activation(r_t[:], ga_ps[:], AF.Relu,
                                         scale=float(aw))
            else:
                nc.scalar.activation(r_t[:], ga_ps[:], AF.Relu)
                r2 = pool.tile([128, S], dtype, tag=tag, bufs=bufs,
                               name=f"r2{b}_{m}")
                nc.vector.tensor_scalar_mul(r2[:], r_t[:], float(aw))
                r_t = r2
            return r_t

        rb_pool = ctx.enter_context(tc.tile_pool(name="rbank", bufs=1))
        for b in range(N_PRE):
            audT = audio_proc(b)
            for m in range(SC):
                bank[("t", b, m)] = relu_gt_raw(b, m, BF16, rb_pool, "tb",
                                                SC * N_PRE)
                bank[("a", b, m)] = relu_ga(b, m, audT, BF16, rb_pool, "rb",
                                            SC * N_PRE)

        # ---------------- per-batch fusion ----------------
        for b in range(BPC):
            audT = None if b < N_PRE else audio_proc(b)
            am_b = smalls.tile([1, S], F32, tag="amb", bufs=1, name=f"am{b}")
            nc.sync.dma_start(am_b[:], am_d[b:b + 1, :])

            # ---- decoupled row-0 softmax chain (tiny ops only) ----
            # row 0 of each gram via a [1,512] matmul against column 0.
            gtr_ps = ps_mm.tile([1, S], F32, tag="proj", name=f"gtr{b}")
            nc.tensor.matmul(gtr_ps[:], textT_all[:, b, 0:1],
                             textT_all[:, b, :])
            rt_row = smalls.tile([1, S], F32, tag="rtrow", bufs=1,
                                 name=f"rtr{b}")
            nc.scalar.activation(rt_row[:], gtr_ps[:], AF.Relu,
                                 scale=inv_w2_r[0:1, 0:1])
            if b >= N_PRE:
                gar_ps = ps_mm.tile([1, S], F32, tag="proj", name=f"gar{b}")
                nc.tensor.matmul(gar_ps[:], audT[:, 0:1], audT[:])
            ra_row = smalls.tile([1, S], F32, tag="rarow", bufs=1,
                                 name=f"rar{b}")
            if b < N_PRE:
                # row 0 of relu(aw*G_a) == row 0 of the banked m=0 tile
                nc.vector.tensor_copy(ra_row[:], bank[("a", b, 0)][0:1, :])
            else:
                if aw >= 0.0:
                    nc.scalar.activation(ra_row[:], gar_ps[:], AF.Relu,
                                         scale=float(aw))
                else:
                    nc.scalar.activation(ra_row[:], gar_ps[:], AF.Relu)
                    nc.vector.tensor_scalar_mul(ra_row[:], ra_row[:],
                                                float(aw))
            z = smalls.tile([1, S], F32, tag="z", bufs=1, name=f"z{b}")
            if tw == 1.0:
                nc.vector.tensor_add(z[:], rt_row[:], ra_row[:])
            else:
                nc.vector.scalar_tensor_tensor(z[:], rt_row[:], float(tw),
                                               ra_row[:], op0=ALU.mult,
                                               op1=ALU.add)
            nc.vector.tensor_add(z[:], z[:], am_b[:])
            mx = smalls.tile([1, 1], F32, tag="mx")
            nc.vector.reduce_max(mx[:], z[:], axis=AX.X)
            mneg = smalls.tile([1, 1], F32, tag="mneg")
            nc.vector.tensor_scalar_mul(mneg[:], mx[:], -1.0)
            e_row = smalls.tile([1, S], F32, tag="erow", bufs=1, name=f"e{b}")
            sumexp = smalls.tile([1, 1], F32, tag="sumexp")
            nc.scalar.activation(e_row[:], z[:], AF.Exp,
                                 bias=mneg[:], accum_out=sumexp[:])
            rcp = smalls.tile([1, 1], F32, tag="rcp")
            nc.vector.reciprocal(rcp[:], sumexp[:])
            rcp_r = smalls.tile([128, 1], F32, tag="rcpr")
            nc.gpsimd.partition_broadcast(rcp_r[:], rcp[:])
            e_rep = work.tile([128, S], F32, tag="erep", bufs=1, name=f"er{b}")
            nc.gpsimd.partition_broadcast(e_rep[:], e_row[:])

            fdnum = smalls.tile([128, HC], F32, tag="fdnum")
            for c in range(HC):
                scr = work.tile([128, S], F32, tag="ttrscr", bufs=1,
                                name=f"scr{b}_{c}")
                nc.vector.scalar_tensor_tensor(
                    scr[:], hidT_all[:, b, c, :].bitcast(F32),
                    1.0, e_rep[:],
                    op0=ALU.mult, op1=ALU.mult,
                    accum_out=fdnum[:, c:c + 1])
            nc.vector.scalar_tensor_tensor(
                fd_all[:, :, b], fdnum[:], rcp_r[:, 0:1],
                hidT_all[:, b, :, 0].bitcast(F32),
                op0=ALU.mult, op1=ALU.add)

            # ---- bulk [128,512] tile pipeline ----
            for m in range(SC):
                idx = b * SC + m
                msl = slice(m * 128, (m + 1) * 128)
                a_t = att.tile([128, S], F32, tag="att", name=f"a{b}_{m}")
                if ("t", b, m) in bank:
                    nc.scalar.activation(a_t[:], bank[("t", b, m)][:],
                                         AF.Copy, scale=inv_w2_r[:])
                else:
                    gt_ps = ps_g.tile([128, S], F32, tag="g", name=f"gt{b}_{m}")
                    nc.tensor.matmul(gt_ps[:], textT_all[:, b, msl],
                                     textT_all[:, b, :])
                    nc.scalar.activation(a_t[:], gt_ps[:], AF.Relu,
                                         scale=inv_w2_r[:])
                nc.sync.dma_start(tatt_d[b, msl, :], a_t[:])

                if ("a", b, m) in bank:
                    r_t = bank[("a", b, m)]
                else:
                    r_t = relu_ga(b, m, audT, F32, att, "att", 4)

                f_pre = att.tile([128, S], F32, tag="att", name=f"f{b}_{m}")
                if tw == 1.0:
                    if idx % 2 == 0:
                        nc.gpsimd.tensor_add(f_pre[:], a_t[:], r_t[:])
                    else:
                        nc.vector.tensor_add(f_pre[:], a_t[:], r_t[:])
                else:
                    nc.vector.scalar_tensor_tensor(
                        f_pre[:], a_t[:], float(tw), r_t[:],
                        op0=ALU.mult, op1=ALU.add)

                if fb == 0.0 and tw >= 0.0 and aw >= 0.0:
                    f_out = f_pre
                elif tw >= 0.0 and aw >= 0.0 and fb >= 0.0:
                    f_out = att.tile([128, S], F32, tag="att", name=f"fo{b}_{m}")
                    nc.scalar.add(f_out[:], f_pre[:], float(fb))
                else:
                    f_out = att.tile([128, S], F32, tag="att", name=f"fo{b}_{m}")
                    fb_t = smalls.tile([128, 1], F32, tag="fbt", bufs=1)
                    nc.gpsimd.memset(fb_t[:], float(fb))
                    nc.scalar.activation(f_out[:], f_pre[:], AF.Relu,
                                         bias=fb_t[:])
                nc.sync.dma_start(fatt_d[b, msl, :], f_out[:])

        # ---------------- dense + layernorm on row-0 states ----------------
        h0a = ps_g.tile([BPC, 512], F32, tag="g")
        h0b = ps_g.tile([BPC, H - 512], F32, tag="g")
        for c in range(HC):
            nc.tensor.matmul(h0a[:], fd_all[:, c, :], dwT[:, c, 0:512],
                             start=(c == 0), stop=(c == HC - 1))
            nc.tensor.matmul(h0b[:], fd_all[:, c, :], dwT[:, c, 512:H],
                             start=(c == 0), stop=(c == HC - 1))
        xb = smalls.tile([BPC, H], F32, tag="xln", bufs=2)
        nc.vector.tensor_add(xb[:, 0:512], h0a[:], db_r[:, 0:512])
        nc.vector.tensor_add(xb[:, 512:H], h0b[:], db_r[:, 512:H])
        usum = smalls.tile([BPC, 1], F32, tag="usum")
        nc.vector.reduce_sum(usum[:], xb[:], axis=AX.X)
        uneg = smalls.tile([BPC, 1], F32, tag="uneg")
        nc.vector.tensor_scalar_mul(uneg[:], usum[:], -1.0 / H)
        xc = smalls.tile([BPC, H], F32, tag="xln", bufs=2)
        nc.vector.tensor_scalar_add(xc[:], xb[:], uneg[:])
        sq2 = smalls.tile([BPC, H], F32, tag="xln", bufs=2)
        v = smalls.tile([BPC, 1], F32, tag="v")
        nc.scalar.activation(sq2[:], xc[:], AF.Square, accum_out=v[:])
        eps_t = smalls.tile([BPC, 1], F32, tag="eps", bufs=1)
        nc.gpsimd.memset(eps_t[:], float(LN_EPS))
        std = smalls.tile([BPC, 1], F32, tag="std")
        nc.scalar.activation(std[:], v[:], AF.Sqrt, scale=1.0 / H,
                             bias=eps_t[:])
        rstd = smalls.tile([BPC, 1], F32, tag="rstd")
        nc.vector.reciprocal(rstd[:], std[:])
        y1 = smalls.tile([BPC, H], F32, tag="xln", bufs=2)
        nc.vector.scalar_tensor_tensor(y1[:], xc[:], rstd[:, 0:1], lnw_r[:],
                                       op0=ALU.mult, op1=ALU.mult)
        y2 = smalls.tile([BPC, H], F32, tag="xln", bufs=2)
        nc.vector.tensor_add(y2[:], y1[:], lnb_r[:])
        nc.sync.dma_start(out0_d[:], y2[:])


# ------------------------------------------------------------------
# NTFF profiling hook (only used when PROFILE=True)
# ------------------------------------------------------------------
def _install_profile_hook():
    try:
        import antenv.axon_hooks  # noqa
        return
    except ImportError:
        pass
    so_path = "/opt/axon/libaxon_pjrt.so"
    try:
        lib = ctypes.CDLL(so_path)
    except OSError:
        return
    if not hasattr(lib, "axon_start_nrt_profile"):
        return
    lib.axon_start_nrt_profile.argtypes = [ctypes.POINTER(ctypes.c_int64),
                                           ctypes.c_size_t]
    lib.axon_start_nrt_profile.restype = ctypes.c_int64
    lib.axon_stop_nrt_profile.argtypes = [ctypes.c_char_p]
    lib.axon_stop_nrt_profile.restype = ctypes.c_int64

    @contextlib.contextmanager
    def _hook(output_dir, device_ids):
        import jax
        jax.devices()
        if device_ids:
            ids = (ctypes.c_int64 * len(device_ids))(*device_ids)
            rc = lib.axon_start_nrt_profile(ids, len(device_ids))
        else:
            rc = lib.axon_start_nrt_profile(None, 0)
        if rc != 0:
            raise RuntimeError(f"axon_start_nrt_profile rc={rc}")
        try:
            yield
        finally:
            n = lib.axon_stop_nrt_profile(str(output_dir).encode())
            if n < 0:
                raise RuntimeError(f"axon_stop_nrt_profile rc={n}")

    mod = types.ModuleType("antenv.axon_hooks")
    _hook_box = [_hook]
    mod.get_axon_ntff_profile_hook = lambda: _hook_box[0]
    mod.set_axon_ntff_profile_hook = lambda h: _hook_box.__setitem__(0, h)
    sys.modules["antenv.axon_hooks"] = mod
    import antenv
    antenv.axon_hooks = mod


# ------------------------------------------------------------------
# Host wrapper
# ------------------------------------------------------------------
def kernel(hidden_states, audio_data, attention_mask, Wt, Wa, text_w, audio_w,
           fbias, dense_W, dense_b, ln_w, ln_b):
    global LAST_EXEC_NS
    hs = np.ascontiguousarray(np.asarray(hidden_states, np.float32))
    ad = np.ascontiguousarray(np.asarray(audio_data, np.float32))
    am = np.ascontiguousarray(
        np.asarray(attention_mask, np.float32).reshape(B, S))
    wt = np.ascontiguousarray(np.asarray(Wt, np.float32))
    wa = np.ascontiguousarray(np.asarray(Wa, np.float32))
    dw = np.ascontiguousarray(np.asarray(dense_W, np.float32))
    db = np.ascontiguousarray(np.asarray(dense_b, np.float32))
    lnw = np.ascontiguousarray(np.asarray(ln_w, np.float32))
    lnb = np.ascontiguousarray(np.asarray(ln_b, np.float32))
    tw = float(np.asarray(text_w).reshape(-1)[0])
    aw = float(np.asarray(audio_w).reshape(-1)[0])
    fb = float(np.asarray(fbias).reshape(-1)[0])

    key = (tw, aw, fb)
    if key not in _compiled:
        _compiled[key] = _build(tw, aw, fb)
    nc = _compiled[key]

    in_maps = []
    for i in range(N_CORES):
        sl = slice(i * BPC, (i + 1) * BPC)
        in_maps.append({
            "hid": hs[sl], "aud": ad[sl], "am": am[sl],
            "wt": wt, "wa": wa, "dw": dw, "db": db,
            "lnw": lnw, "lnb": lnb,
        })

    if PROFILE:
        _install_profile_hook()
    res = run_bass_kernel_spmd(nc, in_maps, list(range(N_CORES)),
                               trace=PROFILE)
    LAST_EXEC_NS = res.exec_time_ns

    h0 = np.concatenate([res.results[i]["out0"] for i in range(N_CORES)], 0)
    t_att = np.concatenate([res.results[i]["t_att"] for i in range(N_CORES)], 0)
    f_att = np.concatenate([res.results[i]["f_att"] for i in range(N_CORES)], 0)
    return h0, t_att, f_att


# revision 18
# speedup vs baseline: 1.1216x; 1.1216x over previous
"""Trainium2 Bass kernel for the multimodal BERT fusion block.

Contract: kernel(**inputs) takes FULL unsharded numpy inputs (as produced by
setup_inputs()), runs an SPMD Bass kernel on 8 NeuronCores (data-parallel over
the batch dim, params replicated), and returns the FULL outputs
(h[:,0], text_att1, fusion_att1) as numpy arrays.

Math per batch b (S=512 tokens, H=768, P=30 proj dim, FD=74 audio feat):
  textT[b]  = Wt @ hidden[b]^T                    [30, 512]
  ssq       = sum(textT^2) over ALL batches       (global -> AllReduce)
  inv_w2    = ssq^-0.5   (w = ssq^0.25; text/w gram scale = 1/w^2)
  text_att1 = relu((textT^T textT) * inv_w2)      [512, 512]   (output)
  audT[b]   = Wa @ audio[b]^T                     [30, 512]
  audio_att = relu(audT^T audT)
  F_pre     = tw*text_att1 + aw*audio_att ; fusion_att1 = relu(F_pre+fb) (output)
  row0: softmax(F_pre[0,:] + am[b,:]) -> e/sum ;  fd0 = e@hidden[b]/sum + hidden[b,0]
  h0 = LN(fd0 @ dW^T + db) * ln_w + ln_b          [768]        (output h[:,0])

Implementation notes:
  - hidden^T is built on-chip via PE transposes (f32 DMA transpose does not
    exist on trn2) and kept resident in fp32r; it feeds both the text
    projection (PE, fp32r) and the softmax-weighted row-0 reduction (DVE
    tensor_tensor_reduce dot products).
  - All large matmuls run in fp32r (1 cycle/row at N>=256 vs 4 for fp32).
  - A tiny scalar AllReduce distributes the global text norm.
"""
import ctypes
import contextlib
import sys
import types

import numpy as np

import concourse.bass as bass
import concourse.bacc as bacc
import concourse.mybir as mybir
import concourse.tile as tile
from concourse import masks
from concourse.bass_utils import run_bass_kernel_spmd

B, S, H, FD, P = 64, 512, 768, 74, 30
N_CORES = 8
BPC = B // N_CORES          # 8 batches per core
HC, SC = H // 128, S // 128  # 6, 4
LN_EPS = 1e-12

F32 = mybir.dt.float32
F32R = mybir.dt.float32r
AF = mybir.ActivationFunctionType
ALU = mybir.AluOpType
AX = mybir.AxisListType

PROFILE = False          # set True (from test.py) to capture HW exec time
LAST_EXEC_NS = None

_compiled = {}


def _build(tw: float, aw: float, fb: float):
    nc = bacc.Bacc("TRN2", target_bir_lowering=False, debug=False,
                   num_devices=N_CORES)

    hid_d = nc.dram_tensor("hid", [BPC, S, H], F32, kind="ExternalInput").ap()
    aud_d = nc.dram_tensor("aud", [BPC, S, FD], F32, kind="ExternalInput").ap()
    am_d = nc.dram_tensor("am", [BPC, S], F32, kind="ExternalInput").ap()
    wt_d = nc.dram_tensor("wt", [P, H], F32, kind="ExternalInput").ap()
    wa_d = nc.dram_tensor("wa", [P, FD], F32, kind="ExternalInput").ap()
    dw_d = nc.dram_tensor("dw", [H, H], F32, kind="ExternalInput").ap()
    db_d = nc.dram_tensor("db", [H], F32, kind="ExternalInput").ap()
    lnw_d = nc.dram_tensor("lnw", [H], F32, kind="ExternalInput").ap()
    lnb_d = nc.dram_tensor("lnb", [H], F32, kind="ExternalInput").ap()

    tatt_d = nc.dram_tensor("t_att", [BPC, S, S], F32, kind="ExternalOutput").ap()
    fatt_d = nc.dram_tensor("f_att", [BPC, S, S], F32, kind="ExternalOutput").ap()
    out0_d = nc.dram_tensor("out0", [BPC, H], F32, kind="ExternalOutput").ap()

    with tile.TileContext(nc) as tc:
        _body(nc, tc, tw, aw, fb,
              hid_d, aud_d, am_d, wt_d, wa_d, dw_d, db_d, lnw_d, lnb_d,
              tatt_d, fatt_d, out0_d)
    nc.compile()
    return nc


def _body(nc, tc, tw, aw, fb,
          hid_d, aud_d, am_d, wt_d, wa_d, dw_d, db_d, lnw_d, lnb_d,
          tatt_d, fatt_d, out0_d):
    from contextlib import ExitStack
    ctx = ExitStack()
    with ctx:
        const = ctx.enter_context(tc.tile_pool(name="const", bufs=1))
        persist = ctx.enter_context(tc.tile_pool(name="persist", bufs=1))
        work = ctx.enter_context(tc.tile_pool(name="work", bufs=2))
        att = ctx.enter_context(tc.tile_pool(name="att", bufs=8))
        smalls = ctx.enter_context(tc.tile_pool(name="smalls", bufs=2))
        dram = ctx.enter_context(tc.tile_pool(name="dram", bufs=1, space="DRAM"))

        ps_tr = ctx.enter_context(tc.tile_pool(name="ps_tr", bufs=3, space="PSUM"))
        ps_mm = ctx.enter_context(tc.tile_pool(name="ps_mm", bufs=2, space="PSUM"))
        ps_g = ctx.enter_context(tc.tile_pool(name="ps_g", bufs=2, space="PSUM"))

        # ---------------- constants / params ----------------
        ident = const.tile([128, 128], F32)
        masks.make_identity(nc, ident[:])

        ones_p = const.tile([P, 1], F32)
        nc.gpsimd.memset(ones_p[:], 1.0)

        # Wt^T: [P, H] -> HC chunks [128, P]
        wt_nat = const.tile([P, H], F32)
        nc.sync.dma_start(wt_nat[:], wt_d[:])
        wtT = const.tile([128, HC, P], F32R)
        for c in range(HC):
            tp = ps_tr.tile([128, P], F32, tag="tr")
            nc.tensor.transpose(tp[:], wt_nat[:, c * 128:(c + 1) * 128],
                                ident[:P, :P])
            nc.scalar.copy(wtT[:, c, :], tp[:])

        # Wa^T: [P, FD] -> [FD, P]
        wa_nat = const.tile([P, FD], F32)
        nc.sync.dma_start(wa_nat[:], wa_d[:])
        waT = const.tile([FD, P], F32R)
        tpw = ps_tr.tile([FD, P], F32, tag="tr")
        nc.tensor.transpose(tpw[:], wa_nat[:], ident[:P, :P])
        nc.scalar.copy(waT[:], tpw[:])

        # ln/db replicated across BPC partitions via broadcast-read DMA
        db_r = const.tile([BPC, H], F32)
        lnw_r = const.tile([BPC, H], F32)
        lnb_r = const.tile([BPC, H], F32)
        for dst, src in ((db_r, db_d), (lnw_r, lnw_d), (lnb_r, lnb_d)):
            s1 = src[:].rearrange("(o h) -> o h", o=1)
            nc.sync.dma_start(dst[:], s1.to_broadcast((BPC, H)))

        # ---------------- persistent state ----------------
        hidT_all = persist.tile([128, BPC, HC, S], F32R)    # hidden^T, fp32r
        textT_all = persist.tile([P, BPC, S], F32R)         # raw text proj
        ssq_all = persist.tile([P, BPC], F32)
        fd_all = persist.tile([128, HC, BPC], F32R)

        # ---------------- phase 1: load hidden, transpose, text proj ----
        ph1_cm = tc.tile_pool(name="ph1", bufs=2)
        ph1 = ph1_cm.__enter__()
        for b in range(BPC):
            for half in range(2):
                hid_nat = ph1.tile([128, 2, H], F32, tag="hidnat", bufs=2,
                                   name=f"hn{b}_{half}")
                for i2 in range(2):
                    i = 2 * half + i2
                    nc.sync.dma_start(hid_nat[:, i2, :],
                                      hid_d[b, i * 128:(i + 1) * 128, :])
                for c in range(HC):
                    for i2 in range(2):
                        i = 2 * half + i2
                        trp = ps_tr.tile([128, 128], F32, tag="tr")
                        nc.tensor.transpose(
                            trp[:], hid_nat[:, i2, c * 128:(c + 1) * 128],
                            ident[:, :])
                        dst = hidT_all[:, b, c, i * 128:(i + 1) * 128]
                        if (c + i) % 2 == 0:
                            nc.scalar.copy(dst, trp[:])
                        else:
                            nc.vector.tensor_copy(dst, trp[:])
            tx_ps = ps_mm.tile([P, S], F32, tag="proj")
            for c in range(HC):
                nc.tensor.matmul(tx_ps[:], wtT[:, c, :], hidT_all[:, b, c, :],
                                 start=(c == 0), stop=(c == HC - 1))
            nc.scalar.copy(textT_all[:, b, :], tx_ps[:])
            sq_ps = ps_mm.tile([P, S], F32, tag="proj")
            nc.scalar.activation(sq_ps[:], tx_ps[:], AF.Square,
                                 accum_out=ssq_all[:, b:b + 1])

        ph1_cm.__exit__(None, None, None)

        # ---------------- global sum-of-squares (AllReduce) ----------------
        ssq_vec = smalls.tile([P, 1], F32, bufs=1)
        nc.vector.reduce_sum(ssq_vec[:], ssq_all[:], axis=AX.X)
        ssq_ps = ps_mm.tile([1, 1], F32, tag="proj")
        nc.tensor.matmul(ssq_ps[:], ones_p[:], ssq_vec[:])
        ssq_sc = smalls.tile([1, 1], F32, bufs=1)
        nc.scalar.copy(ssq_sc[:], ssq_ps[:])
        cc_in = dram.tile([1, 1], F32)
        cc_out = dram.tile([1, 1], F32)
        nc.gpsimd.dma_start(cc_in[:], ssq_sc[:])
        nc.gpsimd.collective_compute(
            "AllReduce", ALU.add,
            replica_groups=[list(range(N_CORES))],
            ins=[cc_in.opt()], outs=[cc_out.opt()])
        ssq_g = smalls.tile([1, 1], F32, bufs=1)
        nc.gpsimd.dma_start(ssq_g[:], cc_out[:])
        w2 = smalls.tile([1, 1], F32, bufs=1)
        nc.scalar.sqrt(w2[:], ssq_g[:])
        inv_w2 = smalls.tile([1, 1], F32, bufs=1)
        nc.vector.reciprocal(inv_w2[:], w2[:])
        inv_w2_r = smalls.tile([128, 1], F32, bufs=1)
        nc.gpsimd.partition_broadcast(inv_w2_r[:], inv_w2[:])

        # ---------------- dW^T build (fills the collective window) --------
        dwT = persist.tile([128, HC, H], F32R)
        for oc in range(HC):
            dw_nat = work.tile([128, H], F32, tag="dwnat", bufs=1)
            nc.sync.dma_start(dw_nat[:], dw_d[oc * 128:(oc + 1) * 128, :])
            for hc in range(HC):
                tp2 = ps_tr.tile([128, 128], F32, tag="tr")
                nc.tensor.transpose(tp2[:], dw_nat[:, hc * 128:(hc + 1) * 128],
                                    ident[:, :])
                if (oc + hc) % 2 == 0:
                    nc.scalar.copy(dwT[:, hc, oc * 128:(oc + 1) * 128], tp2[:])
                else:
                    nc.vector.tensor_copy(dwT[:, hc, oc * 128:(oc + 1) * 128],
                                          tp2[:])

        # ---------------- audio pipeline + gram precompute ----------------
        BF16 = mybir.dt.bfloat16
        # batches whose relu(G_t) / relu(aw*G_a) are precomputed (in bf16)
        # during the collective window; relu commutes with the positive
        # 1/w^2 scale, so relu(G_t) can be built before the AllReduce lands.
        N_PRE = 0  # bank disabled: SBUF spent on pipeline depth instead
        bank = {}

        def audio_proc(b):
            """load+transpose+project audio for batch b -> audT [P, S] f32r"""
            aud_ps = ps_tr.tile([FD, S], F32, tag="atr", bufs=1, name=f"aud_ps{b}")
            for i in range(SC):
                ach = work.tile([128, FD], F32, tag="ach", bufs=2, name=f"ach{b}_{i}")
                nc.sync.dma_start(ach[:], aud_d[b, i * 128:(i + 1) * 128, :])
                nc.tensor.transpose(aud_ps[:, i * 128:(i + 1) * 128],
                                    ach[:], ident[:, :])
            audT_raw = work.tile([FD, S], F32R, tag="audraw", bufs=2, name=f"ar{b}")
            if b % 2 == 0:
                nc.scalar.copy(audT_raw[:], aud_ps[:])
            else:
                nc.vector.tensor_copy(audT_raw[:], aud_ps[:])
            ap_ps = ps_mm.tile([P, S], F32, tag="proj", name=f"ap_ps{b}")
            nc.tensor.matmul(ap_ps[:], waT[:], audT_raw[:])
            audT = work.tile([P, S], F32R, tag="audT", bufs=3, name=f"audT{b}")
            nc.scalar.copy(audT[:], ap_ps[:])
            return audT

        def relu_gt_raw(b, m, dtype, pool, tag, bufs):
            """relu(G_t raw) for (b, m) -> tile (scale by inv_w2 later)"""
            idx = b * SC + m
            msl = slice(m * 128, (m + 1) * 128)
            gt_ps = ps_g.tile([128, S], F32, tag="g", name=f"gt{b}_{m}")
            nc.tensor.matmul(gt_ps[:], textT_all[:, b, msl],
                             textT_all[:, b, :])
            t = pool.tile([128, S], dtype, tag=tag, bufs=bufs, name=f"t{b}_{m}")
            if idx % 2 == 0:
                nc.scalar.activation(t[:], gt_ps[:], AF.Relu)
            else:
                nc.vector.tensor_scalar_max(t[:], gt_ps[:], 0.0)
            return t

        def relu_ga(b, m, audT, dtype, pool, tag, bufs):
            """relu(aw * G_a) for (b, m) -> tile in `pool`"""
            idx = b * SC + m
            msl = slice(m * 128, (m + 1) * 128)
            ga_ps = ps_g.tile([128, S], F32, tag="g", name=f"ga{b}_{m}")
            nc.tensor.matmul(ga_ps[:], audT[:, msl], audT[:])
            r_t = pool.tile([128, S], dtype, tag=tag, bufs=bufs,
                            name=f"r{b}_{m}")
            if aw >= 0.0:
                if aw == 1.0 and idx % 2 == 0:
                    nc.vector.tensor_scalar_max(r_t[:], ga_ps[:], 0.0)
                else:
                    nc.scalar.activation(r_t[:], ga_ps[:], AF.Relu,
                                         scale=float(aw))
            else:
                nc.scalar.activation(r_t[:], ga_ps[:], AF.Relu)
                r2 = pool.tile([128, S], dtype, tag=tag, bufs=bufs,
                               name=f"r2{b}_{m}")
                nc.vector.tensor_scalar_mul(r2[:], r_t[:], float(aw))
                r_t = r2
            return r_t

        rb_pool = ctx.enter_context(tc.tile_pool(name="rbank", bufs=1))
        for b in range(N_PRE):
            audT = audio_proc(b)
            for m in range(SC):
                bank[("t", b, m)] = relu_gt_raw(b, m, BF16, rb_pool, "tb",
                                                SC * N_PRE)
                bank[("a", b, m)] = relu_ga(b, m, audT, BF16, rb_pool, "rb",
                                            SC * N_PRE)

        # ---------------- per-batch fusion ----------------
        for b in range(BPC):
            audT = None if b < N_PRE else audio_proc(b)
            am_b = smalls.tile([1, S], F32, tag="amb", bufs=1, name=f"am{b}")
            nc.sync.dma_start(am_b[:], am_d[b:b + 1, :])

            # ---- decoupled row-0 softmax chain (tiny ops only) ----
            # row 0 of each gram via a [1,512] matmul against column 0.
            gtr_ps = ps_mm.tile([1, S], F32, tag="proj", name=f"gtr{b}")
            nc.tensor.matmul(gtr_ps[:], textT_all[:, b, 0:1],
                             textT_all[:, b, :])
            rt_row = smalls.tile([1, S], F32, tag="rtrow", bufs=1,
                                 name=f"rtr{b}")
            nc.scalar.activation(rt_row[:], gtr_ps[:], AF.Relu,
                                 scale=inv_w2_r[0:1, 0:1])
            if b >= N_PRE:
                gar_ps = ps_mm.tile([1, S], F32, tag="proj", name=f"gar{b}")
                nc.tensor.matmul(gar_ps[:], audT[:, 0:1], audT[:])
            ra_row = smalls.tile([1, S], F32, tag="rarow", bufs=1,
                                 name=f"rar{b}")
            if b < N_PRE:
                # row 0 of relu(aw*G_a) == row 0 of the banked m=0 tile
                nc.vector.tensor_copy(ra_row[:], bank[("a", b, 0)][0:1, :])
            else:
                if aw >= 0.0:
                    nc.scalar.activation(ra_row[:], gar_ps[:], AF.Relu,
                                         scale=float(aw))
                else:
                    nc.scalar.activation(ra_row[:], gar_ps[:], AF.Relu)
                    nc.vector.tensor_scalar_mul(ra_row[:], ra_row[:],
                                                float(aw))
            z = smalls.tile([1, S], F32, tag="z", bufs=1, name=f"z{b}")
            if tw == 1.0:
                nc.vector.tensor_add(z[:], rt_row[:], ra_row[:])
            else:
                nc.vector.scalar_tensor_tensor(z[:], rt_row[:], float(tw),
                                               ra_row[:], op0=ALU.mult,
                                               op1=ALU.add)
            nc.vector.tensor_add(z[:], z[:], am_b[:])
            mx = smalls.tile([1, 1], F32, tag="mx")
            nc.vector.reduce_max(mx[:], z[:], axis=AX.X)
            mneg = smalls.tile([1, 1], F32, tag="mneg")
            nc.vector.tensor_scalar_mul(mneg[:], mx[:], -1.0)
            e_row = smalls.tile([1, S], F32, tag="erow", bufs=1, name=f"e{b}")
            sumexp = smalls.tile([1, 1], F32, tag="sumexp")
            nc.scalar.activation(e_row[:], z[:], AF.Exp,
                                 bias=mneg[:], accum_out=sumexp[:])
            rcp = smalls.tile([1, 1], F32, tag="rcp")
            nc.vector.reciprocal(rcp[:], sumexp[:])
            rcp_r = smalls.tile([128, 1], F32, tag="rcpr")
            nc.gpsimd.partition_broadcast(rcp_r[:], rcp[:])
            e_rep = work.tile([128, S], F32, tag="erep", bufs=1, name=f"er{b}")
            nc.gpsimd.partition_broadcast(e_rep[:], e_row[:])

            fdnum = smalls.tile([128, HC], F32, tag="fdnum")
            for c in range(HC):
                scr = work.tile([128, S], F32, tag="ttrscr", bufs=1,
                                name=f"scr{b}_{c}")
                nc.vector.scalar_tensor_tensor(
                    scr[:], hidT_all[:, b, c, :].bitcast(F32),
                    1.0, e_rep[:],
                    op0=ALU.mult, op1=ALU.mult,
                    accum_out=fdnum[:, c:c + 1])
            nc.vector.scalar_tensor_tensor(
                fd_all[:, :, b], fdnum[:], rcp_r[:, 0:1],
                hidT_all[:, b, :, 0].bitcast(F32),
                op0=ALU.mult, op1=ALU.add)

            # ---- bulk [128,512] tile pipeline ----
            for m in range(SC):
                idx = b * SC + m
                msl = slice(m * 128, (m + 1) * 128)
                a_t = att.tile([128, S], F32, tag="att", name=f"a{b}_{m}")
                if ("t", b, m) in bank:
                    nc.scalar.activation(a_t[:], bank[("t", b, m)][:],
                                         AF.Copy, scale=inv_w2_r[:])
                else:
                    gt_ps = ps_g.tile([128, S], F32, tag="g", name=f"gt{b}_{m}")
                    nc.tensor.matmul(gt_ps[:], textT_all[:, b, msl],
                                     textT_all[:, b, :])
                    nc.scalar.activation(a_t[:], gt_ps[:], AF.Relu,
                                         scale=inv_w2_r[:])
                nc.sync.dma_start(tatt_d[b, msl, :], a_t[:])

                if ("a", b, m) in bank:
                    r_t = bank[("a", b, m)]
                else:
                    r_t = relu_ga(b, m, audT, F32, att, "att", 8)

                f_pre = att.tile([128, S], F32, tag="att", name=f"f{b}_{m}")
                if tw == 1.0:
                    if idx % 2 == 0:
                        nc.gpsimd.tensor_add(f_pre[:], a_t[:], r_t[:])
                    else:
                        nc.vector.tensor_add(f_pre[:], a_t[:], r_t[:])
                else:
                    nc.vector.scalar_tensor_tensor(
                        f_pre[:], a_t[:], float(tw), r_t[:],
                        op0=ALU.mult, op1=ALU.add)

                if fb == 0.0 and tw >= 0.0 and aw >= 0.0:
                    f_out = f_pre
                elif tw >= 0.0 and aw >= 0.0 and fb >= 0.0:
                    f_out = att.tile([128, S], F32, tag="att", name=f"fo{b}_{m}")
                    nc.scalar.add(f_out[:], f_pre[:], float(fb))
                else:
                    f_out = att.tile([128, S], F32, tag="att", name=f"fo{b}_{m}")
                    fb_t = smalls.tile([128, 1], F32, tag="fbt", bufs=1)
                    nc.gpsimd.memset(fb_t[:], float(fb))
                    nc.scalar.activation(f_out[:], f_pre[:], AF.Relu,
                                         bias=fb_t[:])
                nc.sync.dma_start(fatt_d[b, msl, :], f_out[:])

        # ---------------- dense + layernorm on row-0 states ----------------
        h0a = ps_g.tile([BPC, 512], F32, tag="g")
        h0b = ps_g.tile([BPC, H - 512], F32, tag="g")
        for c in range(HC):
            nc.tensor.matmul(h0a[:], fd_all[:, c, :], dwT[:, c, 0:512],
                             start=(c == 0), stop=(c == HC - 1))
            nc.tensor.matmul(h0b[:], fd_all[:, c, :], dwT[:, c, 512:H],
                             start=(c == 0), stop=(c == HC - 1))
        xb = smalls.tile([BPC, H], F32, tag="xln", bufs=2)
        nc.vector.tensor_add(xb[:, 0:512], h0a[:], db_r[:, 0:512])
        nc.vector.tensor_add(xb[:, 512:H], h0b[:], db_r[:, 512:H])
        usum = smalls.tile([BPC, 1], F32, tag="usum")
        nc.vector.reduce_sum(usum[:], xb[:], axis=AX.X)
        uneg = smalls.tile([BPC, 1], F32, tag="uneg")
        nc.vector.tensor_scalar_mul(uneg[:], usum[:], -1.0 / H)
        xc = smalls.tile([BPC, H], F32, tag="xln", bufs=2)
        nc.vector.tensor_scalar_add(xc[:], xb[:], uneg[:])
        sq2 = smalls.tile([BPC, H], F32, tag="xln", bufs=2)
        v = smalls.tile([BPC, 1], F32, tag="v")
        nc.scalar.activation(sq2[:], xc[:], AF.Square, accum_out=v[:])
        eps_t = smalls.tile([BPC, 1], F32, tag="eps", bufs=1)
        nc.gpsimd.memset(eps_t[:], float(LN_EPS))
        std = smalls.tile([BPC, 1], F32, tag="std")
        nc.scalar.activation(std[:], v[:], AF.Sqrt, scale=1.0 / H,
                             bias=eps_t[:])
        rstd = smalls.tile([BPC, 1], F32, tag="rstd")
        nc.vector.reciprocal(rstd[:], std[:])
        y1 = smalls.tile([BPC, H], F32, tag="xln", bufs=2)
        nc.vector.scalar_tensor_tensor(y1[:], xc[:], rstd[:, 0:1], lnw_r[:],
                                       op0=ALU.mult, op1=ALU.mult)
        y2 = smalls.tile([BPC, H], F32, tag="xln", bufs=2)
        nc.vector.tensor_add(y2[:], y1[:], lnb_r[:])
        nc.sync.dma_start(out0_d[:], y2[:])


# ------------------------------------------------------------------
# NTFF profiling hook (only used when PROFILE=True)
# ------------------------------------------------------------------
def _install_profile_hook():
    try:
        import antenv.axon_hooks  # noqa
        return
    except ImportError:
        pass
    so_path = "/opt/axon/libaxon_pjrt.so"
    try:
        lib = ctypes.CDLL(so_path)
    except OSError:
        return
    if not hasattr(lib, "axon_start_nrt_profile"):
        return
    lib.axon_start_nrt_profile.argtypes = [ctypes.POINTER(ctypes.c_int64),
                                           ctypes.c_size_t]
    lib.axon_start_nrt_profile.restype = ctypes.c_int64
    lib.axon_stop_nrt_profile.argtypes = [ctypes.c_char_p]
    lib.axon_stop_nrt_profile.restype = ctypes.c_int64

    @contextlib.contextmanager
    def _hook(output_dir, device_ids):
        import jax
        jax.devices()
        if device_ids:
            ids = (ctypes.c_int64 * len(device_ids))(*device_ids)
            rc = lib.axon_start_nrt_profile(ids, len(device_ids))
        else:
            rc = lib.axon_start_nrt_profile(None, 0)
        if rc != 0:
            raise RuntimeError(f"axon_start_nrt_profile rc={rc}")
        try:
            yield
        finally:
            n = lib.axon_stop_nrt_profile(str(output_dir).encode())
            if n < 0:
                raise RuntimeError(f"axon_stop_nrt_profile rc={n}")

    mod = types.ModuleType("antenv.axon_hooks")
    _hook_box = [_hook]
    mod.get_axon_ntff_profile_hook = lambda: _hook_box[0]
    mod.set_axon_ntff_profile_hook = lambda h: _hook_box.__setitem__(0, h)
    sys.modules["antenv.axon_hooks"] = mod
    import antenv
    antenv.axon_hooks = mod


# ------------------------------------------------------------------
# Host wrapper
# ------------------------------------------------------------------
def kernel(hidden_states, audio_data, attention_mask, Wt, Wa, text_w, audio_w,
           fbias, dense_W, dense_b, ln_w, ln_b):
    global LAST_EXEC_NS
    hs = np.ascontiguousarray(np.asarray(hidden_states, np.float32))
    ad = np.ascontiguousarray(np.asarray(audio_data, np.float32))
    am = np.ascontiguousarray(
        np.asarray(attention_mask, np.float32).reshape(B, S))
    wt = np.ascontiguousarray(np.asarray(Wt, np.float32))
    wa = np.ascontiguousarray(np.asarray(Wa, np.float32))
    dw = np.ascontiguousarray(np.asarray(dense_W, np.float32))
    db = np.ascontiguousarray(np.asarray(dense_b, np.float32))
    lnw = np.ascontiguousarray(np.asarray(ln_w, np.float32))
    lnb = np.ascontiguousarray(np.asarray(ln_b, np.float32))
    tw = float(np.asarray(text_w).reshape(-1)[0])
    aw = float(np.asarray(audio_w).reshape(-1)[0])
    fb = float(np.asarray(fbias).reshape(-1)[0])

    key = (tw, aw, fb)
    if key not in _compiled:
        _compiled[key] = _build(tw, aw, fb)
    nc = _compiled[key]

    in_maps = []
    for i in range(N_CORES):
        sl = slice(i * BPC, (i + 1) * BPC)
        in_maps.append({
            "hid": hs[sl], "aud": ad[sl], "am": am[sl],
            "wt": wt, "wa": wa, "dw": dw, "db": db,
            "lnw": lnw, "lnb": lnb,
        })

    if PROFILE:
        _install_profile_hook()
    res = run_bass_kernel_spmd(nc, in_maps, list(range(N_CORES)),
                               trace=PROFILE)
    LAST_EXEC_NS = res.exec_time_ns

    h0 = np.concatenate([res.results[i]["out0"] for i in range(N_CORES)], 0)
    t_att = np.concatenate([res.results[i]["t_att"] for i in range(N_CORES)], 0)
    f_att = np.concatenate([res.results[i]["f_att"] for i in range(N_CORES)], 0)
    return h0, t_att, f_att


# revision 19
# speedup vs baseline: 1.4179x; 1.2642x over previous
"""Trainium2 Bass kernel for the multimodal BERT fusion block.

Contract: kernel(**inputs) takes FULL unsharded numpy inputs (as produced by
setup_inputs()), runs an SPMD Bass kernel on 8 NeuronCores (data-parallel over
the batch dim, params replicated), and returns the FULL outputs
(h[:,0], text_att1, fusion_att1) as numpy arrays.

Math per batch b (S=512 tokens, H=768, P=30 proj dim, FD=74 audio feat):
  textT[b]  = Wt @ hidden[b]^T                    [30, 512]
  ssq       = sum(textT^2) over ALL batches       (global -> AllReduce)
  inv_w2    = ssq^-0.5   (w = ssq^0.25; text/w gram scale = 1/w^2)
  text_att1 = relu((textT^T textT) * inv_w2)      [512, 512]   (output)
  audT[b]   = Wa @ audio[b]^T                     [30, 512]
  audio_att = relu(audT^T audT)
  F_pre     = tw*text_att1 + aw*audio_att ; fusion_att1 = relu(F_pre+fb) (output)
  row0: softmax(F_pre[0,:] + am[b,:]) -> e/sum ;  fd0 = e@hidden[b]/sum + hidden[b,0]
  h0 = LN(fd0 @ dW^T + db) * ln_w + ln_b          [768]        (output h[:,0])

Implementation notes:
  - hidden^T is built on-chip via PE transposes (f32 DMA transpose does not
    exist on trn2) and kept resident in fp32r; it feeds both the text
    projection (PE, fp32r) and the softmax-weighted row-0 reduction (DVE
    tensor_tensor_reduce dot products).
  - All large matmuls run in fp32r (1 cycle/row at N>=256 vs 4 for fp32).
  - A tiny scalar AllReduce distributes the global text norm.
"""
import ctypes
import contextlib
import sys
import types

import numpy as np

import concourse.bass as bass
import concourse.bacc as bacc
import concourse.mybir as mybir
import concourse.tile as tile
from concourse import masks
from concourse.bass_utils import run_bass_kernel_spmd

B, S, H, FD, P = 64, 512, 768, 74, 30
N_CORES = 8
BPC = B // N_CORES          # 8 batches per core
HC, SC = H // 128, S // 128  # 6, 4
LN_EPS = 1e-12

F32 = mybir.dt.float32
F32R = mybir.dt.float32r
AF = mybir.ActivationFunctionType
ALU = mybir.AluOpType
AX = mybir.AxisListType

PROFILE = False          # set True (from test.py) to capture HW exec time
LAST_EXEC_NS = None

_compiled = {}


def _build(tw: float, aw: float, fb: float):
    nc = bacc.Bacc("TRN2", target_bir_lowering=False, debug=False,
                   num_devices=N_CORES)

    hid_d = nc.dram_tensor("hid", [BPC, S, H], F32, kind="ExternalInput").ap()
    aud_d = nc.dram_tensor("aud", [BPC, S, FD], F32, kind="ExternalInput").ap()
    am_d = nc.dram_tensor("am", [BPC, S], F32, kind="ExternalInput").ap()
    wt_d = nc.dram_tensor("wt", [P, H], F32, kind="ExternalInput").ap()
    wa_d = nc.dram_tensor("wa", [P, FD], F32, kind="ExternalInput").ap()
    dw_d = nc.dram_tensor("dw", [H, H], F32, kind="ExternalInput").ap()
    db_d = nc.dram_tensor("db", [H], F32, kind="ExternalInput").ap()
    lnw_d = nc.dram_tensor("lnw", [H], F32, kind="ExternalInput").ap()
    lnb_d = nc.dram_tensor("lnb", [H], F32, kind="ExternalInput").ap()

    tatt_d = nc.dram_tensor("t_att", [BPC, S, S], F32, kind="ExternalOutput").ap()
    fatt_d = nc.dram_tensor("f_att", [BPC, S, S], F32, kind="ExternalOutput").ap()
    out0_d = nc.dram_tensor("out0", [BPC, H], F32, kind="ExternalOutput").ap()

    with tile.TileContext(nc) as tc:
        _body(nc, tc, tw, aw, fb,
              hid_d, aud_d, am_d, wt_d, wa_d, dw_d, db_d, lnw_d, lnb_d,
              tatt_d, fatt_d, out0_d)
    nc.compile()
    return nc


def _body(nc, tc, tw, aw, fb,
          hid_d, aud_d, am_d, wt_d, wa_d, dw_d, db_d, lnw_d, lnb_d,
          tatt_d, fatt_d, out0_d):
    from contextlib import ExitStack
    ctx = ExitStack()
    with ctx:
        const = ctx.enter_context(tc.tile_pool(name="const", bufs=1))
        persist = ctx.enter_context(tc.tile_pool(name="persist", bufs=1))
        work = ctx.enter_context(tc.tile_pool(name="work", bufs=2))
        att = ctx.enter_context(tc.tile_pool(name="att", bufs=8))
        smalls = ctx.enter_context(tc.tile_pool(name="smalls", bufs=2))
        dram = ctx.enter_context(tc.tile_pool(name="dram", bufs=1, space="DRAM"))

        ps_tr = ctx.enter_context(tc.tile_pool(name="ps_tr", bufs=2, space="PSUM"))
        ps_mm = ctx.enter_context(tc.tile_pool(name="ps_mm", bufs=2, space="PSUM"))
        ps_g = ctx.enter_context(tc.tile_pool(name="ps_g", bufs=3, space="PSUM"))

        # ---------------- constants / params ----------------
        ident = const.tile([128, 128], F32)
        masks.make_identity(nc, ident[:])

        ones_p = const.tile([P, 1], F32)
        nc.gpsimd.memset(ones_p[:], 1.0)

        # Wt^T: [P, H] -> HC chunks [128, P]
        wt_nat = const.tile([P, H], F32)
        nc.sync.dma_start(wt_nat[:], wt_d[:])
        wtT = const.tile([128, HC, P], F32R)
        for c in range(HC):
            tp = ps_tr.tile([128, P], F32, tag="tr")
            nc.tensor.transpose(tp[:], wt_nat[:, c * 128:(c + 1) * 128],
                                ident[:P, :P])
            nc.scalar.copy(wtT[:, c, :], tp[:])

        # Wa^T: [P, FD] -> [FD, P]
        wa_nat = const.tile([P, FD], F32)
        nc.sync.dma_start(wa_nat[:], wa_d[:])
        waT = const.tile([FD, P], F32R)
        tpw = ps_tr.tile([FD, P], F32, tag="tr")
        nc.tensor.transpose(tpw[:], wa_nat[:], ident[:P, :P])
        nc.scalar.copy(waT[:], tpw[:])

        # ln/db replicated across BPC partitions via broadcast-read DMA
        db_r = const.tile([BPC, H], F32)
        lnw_r = const.tile([BPC, H], F32)
        lnb_r = const.tile([BPC, H], F32)
        for dst, src in ((db_r, db_d), (lnw_r, lnw_d), (lnb_r, lnb_d)):
            s1 = src[:].rearrange("(o h) -> o h", o=1)
            nc.sync.dma_start(dst[:], s1.to_broadcast((BPC, H)))

        # ---------------- persistent state ----------------
        hidT_all = persist.tile([128, BPC, HC, S], F32R)    # hidden^T, fp32r
        textT_all = persist.tile([P, BPC, S], F32R)         # raw text proj
        ssq_all = persist.tile([P, BPC], F32)
        fd_all = persist.tile([128, HC, BPC], F32R)

        # ---------------- phase 1: load hidden, transpose, text proj ----
        ph1_cm = tc.tile_pool(name="ph1", bufs=2)
        ph1 = ph1_cm.__enter__()
        for b in range(BPC):
            for half in range(2):
                hid_nat = ph1.tile([128, 2, H], F32, tag="hidnat", bufs=2,
                                   name=f"hn{b}_{half}")
                for i2 in range(2):
                    i = 2 * half + i2
                    nc.sync.dma_start(hid_nat[:, i2, :],
                                      hid_d[b, i * 128:(i + 1) * 128, :])
                for c in range(HC):
                    for i2 in range(2):
                        i = 2 * half + i2
                        trp = ps_tr.tile([128, 128], F32, tag="tr")
                        nc.tensor.transpose(
                            trp[:], hid_nat[:, i2, c * 128:(c + 1) * 128],
                            ident[:, :])
                        dst = hidT_all[:, b, c, i * 128:(i + 1) * 128]
                        if (c + i) % 2 == 0:
                            nc.scalar.copy(dst, trp[:])
                        else:
                            nc.vector.tensor_copy(dst, trp[:])
            tx_ps = ps_mm.tile([P, S], F32, tag="proj")
            for c in range(HC):
                nc.tensor.matmul(tx_ps[:], wtT[:, c, :], hidT_all[:, b, c, :],
                                 start=(c == 0), stop=(c == HC - 1))
            nc.scalar.copy(textT_all[:, b, :], tx_ps[:])
            sq_ps = ps_mm.tile([P, S], F32, tag="proj")
            nc.scalar.activation(sq_ps[:], tx_ps[:], AF.Square,
                                 accum_out=ssq_all[:, b:b + 1])

        ph1_cm.__exit__(None, None, None)

        # ---------------- global sum-of-squares (AllReduce) ----------------
        ssq_vec = smalls.tile([P, 1], F32, bufs=1)
        nc.vector.reduce_sum(ssq_vec[:], ssq_all[:], axis=AX.X)
        ssq_ps = ps_mm.tile([1, 1], F32, tag="proj")
        nc.tensor.matmul(ssq_ps[:], ones_p[:], ssq_vec[:])
        ssq_sc = smalls.tile([1, 1], F32, bufs=1)
        nc.scalar.copy(ssq_sc[:], ssq_ps[:])
        cc_in = dram.tile([1, 1], F32)
        cc_out = dram.tile([1, 1], F32)
        nc.gpsimd.dma_start(cc_in[:], ssq_sc[:])
        nc.gpsimd.collective_compute(
            "AllReduce", ALU.add,
            replica_groups=[list(range(N_CORES))],
            ins=[cc_in.opt()], outs=[cc_out.opt()])
        ssq_g = smalls.tile([1, 1], F32, bufs=1)
        nc.gpsimd.dma_start(ssq_g[:], cc_out[:])
        w2 = smalls.tile([1, 1], F32, bufs=1)
        nc.scalar.sqrt(w2[:], ssq_g[:])
        inv_w2 = smalls.tile([1, 1], F32, bufs=1)
        nc.vector.reciprocal(inv_w2[:], w2[:])
        inv_w2_r = smalls.tile([128, 1], F32, bufs=1)
        nc.gpsimd.partition_broadcast(inv_w2_r[:], inv_w2[:])

        # ---------------- dW^T build (fills the collective window) --------
        dwT = persist.tile([128, HC, H], F32R)
        for oc in range(HC):
            dw_nat = work.tile([128, H], F32, tag="dwnat", bufs=1)
            nc.sync.dma_start(dw_nat[:], dw_d[oc * 128:(oc + 1) * 128, :])
            for hc in range(HC):
                tp2 = ps_tr.tile([128, 128], F32, tag="tr")
                nc.tensor.transpose(tp2[:], dw_nat[:, hc * 128:(hc + 1) * 128],
                                    ident[:, :])
                if (oc + hc) % 2 == 0:
                    nc.scalar.copy(dwT[:, hc, oc * 128:(oc + 1) * 128], tp2[:])
                else:
                    nc.vector.tensor_copy(dwT[:, hc, oc * 128:(oc + 1) * 128],
                                          tp2[:])

        # ---------------- audio pipeline + gram precompute ----------------
        BF16 = mybir.dt.bfloat16
        # batches whose relu(G_t) / relu(aw*G_a) are precomputed (in bf16)
        # during the collective window; relu commutes with the positive
        # 1/w^2 scale, so relu(G_t) can be built before the AllReduce lands.
        N_PRE = 0  # bank disabled: SBUF spent on pipeline depth instead
        bank = {}

        def audio_proc(b):
            """load+transpose+project audio for batch b -> audT [P, S] f32r"""
            aud_ps = ps_tr.tile([FD, S], F32, tag="atr", bufs=1, name=f"aud_ps{b}")
            for i in range(SC):
                ach = work.tile([128, FD], F32, tag="ach", bufs=2, name=f"ach{b}_{i}")
                nc.sync.dma_start(ach[:], aud_d[b, i * 128:(i + 1) * 128, :])
                nc.tensor.transpose(aud_ps[:, i * 128:(i + 1) * 128],
                                    ach[:], ident[:, :])
            audT_raw = work.tile([FD, S], F32R, tag="audraw", bufs=2, name=f"ar{b}")
            if b % 2 == 0:
                nc.scalar.copy(audT_raw[:], aud_ps[:])
            else:
                nc.vector.tensor_copy(audT_raw[:], aud_ps[:])
            ap_ps = ps_mm.tile([P, S], F32, tag="proj", name=f"ap_ps{b}")
            nc.tensor.matmul(ap_ps[:], waT[:], audT_raw[:])
            audT = work.tile([P, S], F32R, tag="audT", bufs=3, name=f"audT{b}")
            nc.scalar.copy(audT[:], ap_ps[:])
            return audT

        def relu_gt_raw(b, m, dtype, pool, tag, bufs):
            """relu(G_t raw) for (b, m) -> tile (scale by inv_w2 later)"""
            idx = b * SC + m
            msl = slice(m * 128, (m + 1) * 128)
            gt_ps = ps_g.tile([128, S], F32, tag="g", name=f"gt{b}_{m}")
            nc.tensor.matmul(gt_ps[:], textT_all[:, b, msl],
                             textT_all[:, b, :])
            t = pool.tile([128, S], dtype, tag=tag, bufs=bufs, name=f"t{b}_{m}")
            if idx % 2 == 0:
                nc.scalar.activation(t[:], gt_ps[:], AF.Relu)
            else:
                nc.vector.tensor_scalar_max(t[:], gt_ps[:], 0.0)
            return t

        def relu_ga(b, m, audT, dtype, pool, tag, bufs):
            """relu(aw * G_a) for (b, m) -> tile in `pool`"""
            idx = b * SC + m
            msl = slice(m * 128, (m + 1) * 128)
            ga_ps = ps_g.tile([128, S], F32, tag="g", name=f"ga{b}_{m}")
            nc.tensor.matmul(ga_ps[:], audT[:, msl], audT[:])
            r_t = pool.tile([128, S], dtype, tag=tag, bufs=bufs,
                            name=f"r{b}_{m}")
            if aw >= 0.0:
                if aw == 1.0 and idx % 2 == 0:
                    nc.vector.tensor_scalar_max(r_t[:], ga_ps[:], 0.0)
                else:
                    nc.scalar.activation(r_t[:], ga_ps[:], AF.Relu,
                                         scale=float(aw))
            else:
                nc.scalar.activation(r_t[:], ga_ps[:], AF.Relu)
                r2 = pool.tile([128, S], dtype, tag=tag, bufs=bufs,
                               name=f"r2{b}_{m}")
                nc.vector.tensor_scalar_mul(r2[:], r_t[:], float(aw))
                r_t = r2
            return r_t

        rb_pool = ctx.enter_context(tc.tile_pool(name="rbank", bufs=1))
        for b in range(N_PRE):
            audT = audio_proc(b)
            for m in range(SC):
                bank[("t", b, m)] = relu_gt_raw(b, m, BF16, rb_pool, "tb",
                                                SC * N_PRE)
                bank[("a", b, m)] = relu_ga(b, m, audT, BF16, rb_pool, "rb",
                                            SC * N_PRE)

        # ---------------- per-batch fusion ----------------
        for b in range(BPC):
            audT = None if b < N_PRE else audio_proc(b)
            am_b = smalls.tile([1, S], F32, tag="amb", bufs=1, name=f"am{b}")
            nc.sync.dma_start(am_b[:], am_d[b:b + 1, :])

            # ---- decoupled row-0 softmax chain (tiny ops only) ----
            # row 0 of each gram via a [1,512] matmul against column 0.
            gtr_ps = ps_mm.tile([1, S], F32, tag="proj", name=f"gtr{b}")
            nc.tensor.matmul(gtr_ps[:], textT_all[:, b, 0:1],
                             textT_all[:, b, :])
            rt_row = smalls.tile([1, S], F32, tag="rtrow", bufs=1,
                                 name=f"rtr{b}")
            nc.scalar.activation(rt_row[:], gtr_ps[:], AF.Relu,
                                 scale=inv_w2_r[0:1, 0:1])
            if b >= N_PRE:
                gar_ps = ps_mm.tile([1, S], F32, tag="proj", name=f"gar{b}")
                nc.tensor.matmul(gar_ps[:], audT[:, 0:1], audT[:])
            ra_row = smalls.tile([1, S], F32, tag="rarow", bufs=1,
                                 name=f"rar{b}")
            if b < N_PRE:
                # row 0 of relu(aw*G_a) == row 0 of the banked m=0 tile
                nc.vector.tensor_copy(ra_row[:], bank[("a", b, 0)][0:1, :])
            else:
                if aw >= 0.0:
                    nc.scalar.activation(ra_row[:], gar_ps[:], AF.Relu,
                                         scale=float(aw))
                else:
                    nc.scalar.activation(ra_row[:], gar_ps[:], AF.Relu)
                    nc.vector.tensor_scalar_mul(ra_row[:], ra_row[:],
                                                float(aw))
            z = smalls.tile([1, S], F32, tag="z", bufs=1, name=f"z{b}")
            if tw == 1.0:
                nc.vector.tensor_add(z[:], rt_row[:], ra_row[:])
            else:
                nc.vector.scalar_tensor_tensor(z[:], rt_row[:], float(tw),
                                               ra_row[:], op0=ALU.mult,
                                               op1=ALU.add)
            nc.vector.tensor_add(z[:], z[:], am_b[:])
            mx = smalls.tile([1, 1], F32, tag="mx")
            nc.vector.reduce_max(mx[:], z[:], axis=AX.X)
            mneg = smalls.tile([1, 1], F32, tag="mneg")
            nc.vector.tensor_scalar_mul(mneg[:], mx[:], -1.0)
            e_row = smalls.tile([1, S], F32, tag="erow", bufs=1, name=f"e{b}")
            sumexp = smalls.tile([1, 1], F32, tag="sumexp")
            nc.scalar.activation(e_row[:], z[:], AF.Exp,
                                 bias=mneg[:], accum_out=sumexp[:])
            rcp = smalls.tile([1, 1], F32, tag="rcp")
            nc.vector.reciprocal(rcp[:], sumexp[:])
            rcp_r = smalls.tile([128, 1], F32, tag="rcpr")
            nc.gpsimd.partition_broadcast(rcp_r[:], rcp[:])
            e_rep = work.tile([128, S], F32, tag="erep", bufs=1, name=f"er{b}")
            nc.gpsimd.partition_broadcast(e_rep[:], e_row[:])

            fdnum = smalls.tile([128, HC], F32, tag="fdnum")
            for c in range(HC):
                scr = work.tile([128, S], F32, tag="ttrscr", bufs=1,
                                name=f"scr{b}_{c}")
                nc.vector.scalar_tensor_tensor(
                    scr[:], hidT_all[:, b, c, :].bitcast(F32),
                    1.0, e_rep[:],
                    op0=ALU.mult, op1=ALU.mult,
                    accum_out=fdnum[:, c:c + 1])
            nc.vector.scalar_tensor_tensor(
                fd_all[:, :, b], fdnum[:], rcp_r[:, 0:1],
                hidT_all[:, b, :, 0].bitcast(F32),
                op0=ALU.mult, op1=ALU.add)

            # ---- bulk [128,512] tile pipeline ----
            for m in range(SC):
                idx = b * SC + m
                msl = slice(m * 128, (m + 1) * 128)
                a_t = att.tile([128, S], F32, tag="att", name=f"a{b}_{m}")
                if ("t", b, m) in bank:
                    nc.scalar.activation(a_t[:], bank[("t", b, m)][:],
                                         AF.Copy, scale=inv_w2_r[:])
                else:
                    gt_ps = ps_g.tile([128, S], F32, tag="g", name=f"gt{b}_{m}")
                    nc.tensor.matmul(gt_ps[:], textT_all[:, b, msl],
                                     textT_all[:, b, :])
                    nc.scalar.activation(a_t[:], gt_ps[:], AF.Relu,
                                         scale=inv_w2_r[:])
                nc.sync.dma_start(tatt_d[b, msl, :], a_t[:])

                if ("a", b, m) in bank:
                    r_t = bank[("a", b, m)]
                else:
                    r_t = relu_ga(b, m, audT, F32, att, "att", 8)

                f_pre = att.tile([128, S], F32, tag="att", name=f"f{b}_{m}")
                if tw == 1.0:
                    nc.vector.tensor_add(f_pre[:], a_t[:], r_t[:])
                else:
                    nc.vector.scalar_tensor_tensor(
                        f_pre[:], a_t[:], float(tw), r_t[:],
                        op0=ALU.mult, op1=ALU.add)

                if fb == 0.0 and tw >= 0.0 and aw >= 0.0:
                    f_out = f_pre
                elif tw >= 0.0 and aw >= 0.0 and fb >= 0.0:
                    f_out = att.tile([128, S], F32, tag="att", name=f"fo{b}_{m}")
                    nc.scalar.add(f_out[:], f_pre[:], float(fb))
                else:
                    f_out = att.tile([128, S], F32, tag="att", name=f"fo{b}_{m}")
                    fb_t = smalls.tile([128, 1], F32, tag="fbt", bufs=1)
                    nc.gpsimd.memset(fb_t[:], float(fb))
                    nc.scalar.activation(f_out[:], f_pre[:], AF.Relu,
                                         bias=fb_t[:])
                nc.sync.dma_start(fatt_d[b, msl, :], f_out[:])

        # ---------------- dense + layernorm on row-0 states ----------------
        h0a = ps_g.tile([BPC, 512], F32, tag="g")
        h0b = ps_g.tile([BPC, H - 512], F32, tag="g")
        for c in range(HC):
            nc.tensor.matmul(h0a[:], fd_all[:, c, :], dwT[:, c, 0:512],
                             start=(c == 0), stop=(c == HC - 1))
            nc.tensor.matmul(h0b[:], fd_all[:, c, :], dwT[:, c, 512:H],
                             start=(c == 0), stop=(c == HC - 1))
        xb = smalls.tile([BPC, H], F32, tag="xln", bufs=2)
        nc.vector.tensor_add(xb[:, 0:512], h0a[:], db_r[:, 0:512])
        nc.vector.tensor_add(xb[:, 512:H], h0b[:], db_r[:, 512:H])
        usum = smalls.tile([BPC, 1], F32, tag="usum")
        nc.vector.reduce_sum(usum[:], xb[:], axis=AX.X)
        uneg = smalls.tile([BPC, 1], F32, tag="uneg")
        nc.vector.tensor_scalar_mul(uneg[:], usum[:], -1.0 / H)
        xc = smalls.tile([BPC, H], F32, tag="xln", bufs=2)
        nc.vector.tensor_scalar_add(xc[:], xb[:], uneg[:])
        sq2 = smalls.tile([BPC, H], F32, tag="xln", bufs=2)
        v = smalls.tile([BPC, 1], F32, tag="v")
        nc.scalar.activation(sq2[:], xc[:], AF.Square, accum_out=v[:])
        eps_t = smalls.tile([BPC, 1], F32, tag="eps", bufs=1)
        nc.gpsimd.memset(eps_t[:], float(LN_EPS))
        std = smalls.tile([BPC, 1], F32, tag="std")
        nc.scalar.activation(std[:], v[:], AF.Sqrt, scale=1.0 / H,
                             bias=eps_t[:])
        rstd = smalls.tile([BPC, 1], F32, tag="rstd")
        nc.vector.reciprocal(rstd[:], std[:])
        y1 = smalls.tile([BPC, H], F32, tag="xln", bufs=2)
        nc.vector.scalar_tensor_tensor(y1[:], xc[:], rstd[:, 0:1], lnw_r[:],
                                       op0=ALU.mult, op1=ALU.mult)
        y2 = smalls.tile([BPC, H], F32, tag="xln", bufs=2)
        nc.vector.tensor_add(y2[:], y1[:], lnb_r[:])
        nc.sync.dma_start(out0_d[:], y2[:])


# ------------------------------------------------------------------
# NTFF profiling hook (only used when PROFILE=True)
# ------------------------------------------------------------------
def _install_profile_hook():
    try:
        import antenv.axon_hooks  # noqa
        return
    except ImportError:
        pass
    so_path = "/opt/axon/libaxon_pjrt.so"
    try:
        lib = ctypes.CDLL(so_path)
    except OSError:
        return
    if not hasattr(lib, "axon_start_nrt_profile"):
        return
    lib.axon_start_nrt_profile.argtypes = [ctypes.POINTER(ctypes.c_int64),
                                           ctypes.c_size_t]
    lib.axon_start_nrt_profile.restype = ctypes.c_int64
    lib.axon_stop_nrt_profile.argtypes = [ctypes.c_char_p]
    lib.axon_stop_nrt_profile.restype = ctypes.c_int64

    @contextlib.contextmanager
    def _hook(output_dir, device_ids):
        import jax
        jax.devices()
        if device_ids:
            ids = (ctypes.c_int64 * len(device_ids))(*device_ids)
            rc = lib.axon_start_nrt_profile(ids, len(device_ids))
        else:
            rc = lib.axon_start_nrt_profile(None, 0)
        if rc != 0:
            raise RuntimeError(f"axon_start_nrt_profile rc={rc}")
        try:
            yield
        finally:
            n = lib.axon_stop_nrt_profile(str(output_dir).encode())
            if n < 0:
                raise RuntimeError(f"axon_stop_nrt_profile rc={n}")

    mod = types.ModuleType("antenv.axon_hooks")
    _hook_box = [_hook]
    mod.get_axon_ntff_profile_hook = lambda: _hook_box[0]
    mod.set_axon_ntff_profile_hook = lambda h: _hook_box.__setitem__(0, h)
    sys.modules["antenv.axon_hooks"] = mod
    import antenv
    antenv.axon_hooks = mod


# ------------------------------------------------------------------
# Host wrapper
# ------------------------------------------------------------------
def kernel(hidden_states, audio_data, attention_mask, Wt, Wa, text_w, audio_w,
           fbias, dense_W, dense_b, ln_w, ln_b):
    global LAST_EXEC_NS
    hs = np.ascontiguousarray(np.asarray(hidden_states, np.float32))
    ad = np.ascontiguousarray(np.asarray(audio_data, np.float32))
    am = np.ascontiguousarray(
        np.asarray(attention_mask, np.float32).reshape(B, S))
    wt = np.ascontiguousarray(np.asarray(Wt, np.float32))
    wa = np.ascontiguousarray(np.asarray(Wa, np.float32))
    dw = np.ascontiguousarray(np.asarray(dense_W, np.float32))
    db = np.ascontiguousarray(np.asarray(dense_b, np.float32))
    lnw = np.ascontiguousarray(np.asarray(ln_w, np.float32))
    lnb = np.ascontiguousarray(np.asarray(ln_b, np.float32))
    tw = float(np.asarray(text_w).reshape(-1)[0])
    aw = float(np.asarray(audio_w).reshape(-1)[0])
    fb = float(np.asarray(fbias).reshape(-1)[0])

    key = (tw, aw, fb)
    if key not in _compiled:
        _compiled[key] = _build(tw, aw, fb)
    nc = _compiled[key]

    in_maps = []
    for i in range(N_CORES):
        sl = slice(i * BPC, (i + 1) * BPC)
        in_maps.append({
            "hid": hs[sl], "aud": ad[sl], "am": am[sl],
            "wt": wt, "wa": wa, "dw": dw, "db": db,
            "lnw": lnw, "lnb": lnb,
        })

    if PROFILE:
        _install_profile_hook()
    res = run_bass_kernel_spmd(nc, in_maps, list(range(N_CORES)),
                               trace=PROFILE)
    LAST_EXEC_NS = res.exec_time_ns

    h0 = np.concatenate([res.results[i]["out0"] for i in range(N_CORES)], 0)
    t_att = np.concatenate([res.results[i]["t_att"] for i in range(N_CORES)], 0)
    f_att = np.concatenate([res.results[i]["f_att"] for i in range(N_CORES)], 0)
    return h0, t_att, f_att


# revision 21
# speedup vs baseline: 1.4356x; 1.0125x over previous
"""Trainium2 Bass kernel for the multimodal BERT fusion block.

Contract: kernel(**inputs) takes FULL unsharded numpy inputs (as produced by
setup_inputs()), runs an SPMD Bass kernel on 8 NeuronCores (data-parallel over
the batch dim, params replicated), and returns the FULL outputs
(h[:,0], text_att1, fusion_att1) as numpy arrays.

Math per batch b (S=512 tokens, H=768, P=30 proj dim, FD=74 audio feat):
  textT[b]  = Wt @ hidden[b]^T                    [30, 512]
  ssq       = sum(textT^2) over ALL batches       (global -> AllReduce)
  inv_w2    = ssq^-0.5   (w = ssq^0.25; text/w gram scale = 1/w^2)
  text_att1 = relu((textT^T textT) * inv_w2)      [512, 512]   (output)
  audT[b]   = Wa @ audio[b]^T                     [30, 512]
  audio_att = relu(audT^T audT)
  F_pre     = tw*text_att1 + aw*audio_att ; fusion_att1 = relu(F_pre+fb) (output)
  row0: softmax(F_pre[0,:] + am[b,:]) -> e/sum ;  fd0 = e@hidden[b]/sum + hidden[b,0]
  h0 = LN(fd0 @ dW^T + db) * ln_w + ln_b          [768]        (output h[:,0])

Implementation notes:
  - hidden^T is built on-chip via PE transposes (f32 DMA transpose does not
    exist on trn2) and kept resident in fp32r; it feeds both the text
    projection (PE, fp32r) and the softmax-weighted row-0 reduction (DVE
    tensor_tensor_reduce dot products).
  - All large matmuls run in fp32r (1 cycle/row at N>=256 vs 4 for fp32).
  - A tiny scalar AllReduce distributes the global text norm.
"""
import ctypes
import contextlib
import sys
import types

import numpy as np

import concourse.bass as bass
import concourse.bacc as bacc
import concourse.mybir as mybir
import concourse.tile as tile
from concourse import masks
from concourse.bass_utils import run_bass_kernel_spmd

B, S, H, FD, P = 64, 512, 768, 74, 30
N_CORES = 8
BPC = B // N_CORES          # 8 batches per core
HC, SC = H // 128, S // 128  # 6, 4
LN_EPS = 1e-12

F32 = mybir.dt.float32
F32R = mybir.dt.float32r
AF = mybir.ActivationFunctionType
ALU = mybir.AluOpType
AX = mybir.AxisListType

PROFILE = False          # set True (from test.py) to capture HW exec time
LAST_EXEC_NS = None

_compiled = {}


def _build(tw: float, aw: float, fb: float):
    nc = bacc.Bacc("TRN2", target_bir_lowering=False, debug=False,
                   num_devices=N_CORES)

    hid_d = nc.dram_tensor("hid", [BPC, S, H], F32, kind="ExternalInput").ap()
    aud_d = nc.dram_tensor("aud", [BPC, S, FD], F32, kind="ExternalInput").ap()
    am_d = nc.dram_tensor("am", [BPC, S], F32, kind="ExternalInput").ap()
    wt_d = nc.dram_tensor("wt", [P, H], F32, kind="ExternalInput").ap()
    wa_d = nc.dram_tensor("wa", [P, FD], F32, kind="ExternalInput").ap()
    dw_d = nc.dram_tensor("dw", [H, H], F32, kind="ExternalInput").ap()
    db_d = nc.dram_tensor("db", [H], F32, kind="ExternalInput").ap()
    lnw_d = nc.dram_tensor("lnw", [H], F32, kind="ExternalInput").ap()
    lnb_d = nc.dram_tensor("lnb", [H], F32, kind="ExternalInput").ap()

    tatt_d = nc.dram_tensor("t_att", [BPC, S, S], F32, kind="ExternalOutput").ap()
    fatt_d = nc.dram_tensor("f_att", [BPC, S, S], F32, kind="ExternalOutput").ap()
    out0_d = nc.dram_tensor("out0", [BPC, H], F32, kind="ExternalOutput").ap()

    with tile.TileContext(nc) as tc:
        _body(nc, tc, tw, aw, fb,
              hid_d, aud_d, am_d, wt_d, wa_d, dw_d, db_d, lnw_d, lnb_d,
              tatt_d, fatt_d, out0_d)
    nc.compile()
    return nc


def _body(nc, tc, tw, aw, fb,
          hid_d, aud_d, am_d, wt_d, wa_d, dw_d, db_d, lnw_d, lnb_d,
          tatt_d, fatt_d, out0_d):
    from contextlib import ExitStack
    ctx = ExitStack()
    with ctx:
        const = ctx.enter_context(tc.tile_pool(name="const", bufs=1))
        persist = ctx.enter_context(tc.tile_pool(name="persist", bufs=1))
        work = ctx.enter_context(tc.tile_pool(name="work", bufs=2))
        att = ctx.enter_context(tc.tile_pool(name="att", bufs=8))
        smalls = ctx.enter_context(tc.tile_pool(name="smalls", bufs=2))
        dram = ctx.enter_context(tc.tile_pool(name="dram", bufs=1, space="DRAM"))

        ps_tr = ctx.enter_context(tc.tile_pool(name="ps_tr", bufs=3, space="PSUM"))
        ps_mm = ctx.enter_context(tc.tile_pool(name="ps_mm", bufs=1, space="PSUM"))
        ps_g = ctx.enter_context(tc.tile_pool(name="ps_g", bufs=3, space="PSUM"))

        # ---------------- constants / params ----------------
        ident = const.tile([128, 128], F32)
        masks.make_identity(nc, ident[:])

        ones_p = const.tile([P, 1], F32)
        nc.gpsimd.memset(ones_p[:], 1.0)

        # Wt^T: [P, H] -> HC chunks [128, P]
        wt_nat = const.tile([P, H], F32)
        nc.sync.dma_start(wt_nat[:], wt_d[:])
        wtT = const.tile([128, HC, P], F32R)
        for c in range(HC):
            tp = ps_tr.tile([128, P], F32, tag="tr")
            nc.tensor.transpose(tp[:], wt_nat[:, c * 128:(c + 1) * 128],
                                ident[:P, :P])
            nc.scalar.copy(wtT[:, c, :], tp[:])

        # Wa^T: [P, FD] -> [FD, P]
        wa_nat = const.tile([P, FD], F32)
        nc.sync.dma_start(wa_nat[:], wa_d[:])
        waT = const.tile([FD, P], F32R)
        tpw = ps_tr.tile([FD, P], F32, tag="tr")
        nc.tensor.transpose(tpw[:], wa_nat[:], ident[:P, :P])
        nc.scalar.copy(waT[:], tpw[:])

        # ln/db replicated across BPC partitions via broadcast-read DMA
        db_r = const.tile([BPC, H], F32)
        lnw_r = const.tile([BPC, H], F32)
        lnb_r = const.tile([BPC, H], F32)
        for dst, src in ((db_r, db_d), (lnw_r, lnw_d), (lnb_r, lnb_d)):
            s1 = src[:].rearrange("(o h) -> o h", o=1)
            nc.sync.dma_start(dst[:], s1.to_broadcast((BPC, H)))

        # ---------------- persistent state ----------------
        hidT_all = persist.tile([128, BPC, HC, S], F32R)    # hidden^T, fp32r
        textT_all = persist.tile([P, BPC, S], F32R)         # raw text proj
        ssq_all = persist.tile([P, BPC], F32)
        fd_all = persist.tile([128, HC, BPC], F32R)

        # ---------------- phase 1: load hidden, transpose, text proj ----
        ph1_cm = tc.tile_pool(name="ph1", bufs=2)
        ph1 = ph1_cm.__enter__()
        for b in range(BPC):
            for half in range(2):
                hid_nat = ph1.tile([128, 2, H], F32, tag="hidnat", bufs=2,
                                   name=f"hn{b}_{half}")
                for i2 in range(2):
                    i = 2 * half + i2
                    nc.sync.dma_start(hid_nat[:, i2, :],
                                      hid_d[b, i * 128:(i + 1) * 128, :])
                for c in range(HC):
                    for i2 in range(2):
                        i = 2 * half + i2
                        trp = ps_tr.tile([128, 128], F32, tag="tr")
                        nc.tensor.transpose(
                            trp[:], hid_nat[:, i2, c * 128:(c + 1) * 128],
                            ident[:, :])
                        dst = hidT_all[:, b, c, i * 128:(i + 1) * 128]
                        if (c + i) % 2 == 0:
                            nc.scalar.copy(dst, trp[:])
                        else:
                            nc.vector.tensor_copy(dst, trp[:])
            tx_ps = ps_mm.tile([P, S], F32, tag="proj")
            for c in range(HC):
                nc.tensor.matmul(tx_ps[:], wtT[:, c, :], hidT_all[:, b, c, :],
                                 start=(c == 0), stop=(c == HC - 1))
            nc.scalar.copy(textT_all[:, b, :], tx_ps[:])
            sq_sb = work.tile([P, S], F32, tag="sqsb", bufs=1, name=f"sq{b}")
            nc.scalar.activation(sq_sb[:], tx_ps[:], AF.Square,
                                 accum_out=ssq_all[:, b:b + 1])

        ph1_cm.__exit__(None, None, None)

        # ---------------- global sum-of-squares (AllReduce) ----------------
        ssq_vec = smalls.tile([P, 1], F32, bufs=1)
        nc.vector.reduce_sum(ssq_vec[:], ssq_all[:], axis=AX.X)
        ssq_ps = ps_mm.tile([1, 1], F32, tag="proj")
        nc.tensor.matmul(ssq_ps[:], ones_p[:], ssq_vec[:])
        ssq_sc = smalls.tile([1, 1], F32, bufs=1)
        nc.scalar.copy(ssq_sc[:], ssq_ps[:])
        cc_in = dram.tile([1, 1], F32)
        cc_out = dram.tile([1, 1], F32)
        nc.gpsimd.dma_start(cc_in[:], ssq_sc[:])
        nc.gpsimd.collective_compute(
            "AllReduce", ALU.add,
            replica_groups=[list(range(N_CORES))],
            ins=[cc_in.opt()], outs=[cc_out.opt()])
        ssq_g = smalls.tile([1, 1], F32, bufs=1)
        nc.gpsimd.dma_start(ssq_g[:], cc_out[:])
        w2 = smalls.tile([1, 1], F32, bufs=1)
        nc.scalar.sqrt(w2[:], ssq_g[:])
        inv_w2 = smalls.tile([1, 1], F32, bufs=1)
        nc.vector.reciprocal(inv_w2[:], w2[:])
        inv_w2_r = smalls.tile([128, 1], F32, bufs=1)
        nc.gpsimd.partition_broadcast(inv_w2_r[:], inv_w2[:])

        # ---------------- dW^T build (fills the collective window) --------
        dwT = persist.tile([128, HC, H], F32R)
        for oc in range(HC):
            dw_nat = work.tile([128, H], F32, tag="dwnat", bufs=1)
            nc.sync.dma_start(dw_nat[:], dw_d[oc * 128:(oc + 1) * 128, :])
            for hc in range(HC):
                tp2 = ps_tr.tile([128, 128], F32, tag="tr")
                nc.tensor.transpose(tp2[:], dw_nat[:, hc * 128:(hc + 1) * 128],
                                    ident[:, :])
                if (oc + hc) % 2 == 0:
                    nc.scalar.copy(dwT[:, hc, oc * 128:(oc + 1) * 128], tp2[:])
                else:
                    nc.vector.tensor_copy(dwT[:, hc, oc * 128:(oc + 1) * 128],
                                          tp2[:])

        # ---------------- audio pipeline + gram precompute ----------------
        BF16 = mybir.dt.bfloat16
        # batches whose relu(G_t) / relu(aw*G_a) are precomputed (in bf16)
        # during the collective window; relu commutes with the positive
        # 1/w^2 scale, so relu(G_t) can be built before the AllReduce lands.
        N_PRE = 0  # bank disabled: SBUF spent on pipeline depth instead
        bank = {}

        def audio_proc(b):
            """load+transpose+project audio for batch b -> audT [P, S] f32r"""
            aud_ps = ps_tr.tile([FD, S], F32, tag="atr", bufs=1, name=f"aud_ps{b}")
            for i in range(SC):
                ach = work.tile([128, FD], F32, tag="ach", bufs=2, name=f"ach{b}_{i}")
                nc.sync.dma_start(ach[:], aud_d[b, i * 128:(i + 1) * 128, :])
                nc.tensor.transpose(aud_ps[:, i * 128:(i + 1) * 128],
                                    ach[:], ident[:, :])
            audT_raw = work.tile([FD, S], F32R, tag="audraw", bufs=2, name=f"ar{b}")
            if b % 2 == 0:
                nc.scalar.copy(audT_raw[:], aud_ps[:])
            else:
                nc.vector.tensor_copy(audT_raw[:], aud_ps[:])
            ap_ps = ps_mm.tile([P, S], F32, tag="proj", name=f"ap_ps{b}")
            nc.tensor.matmul(ap_ps[:], waT[:], audT_raw[:])
            audT = work.tile([P, S], F32R, tag="audT", bufs=2, name=f"audT{b}")
            nc.scalar.copy(audT[:], ap_ps[:])
            return audT

        def relu_gt_raw(b, m, dtype, pool, tag, bufs):
            """relu(G_t raw) for (b, m) -> tile (scale by inv_w2 later)"""
            idx = b * SC + m
            msl = slice(m * 128, (m + 1) * 128)
            gt_ps = ps_g.tile([128, S], F32, tag="g", name=f"gt{b}_{m}")
            nc.tensor.matmul(gt_ps[:], textT_all[:, b, msl],
                             textT_all[:, b, :])
            t = pool.tile([128, S], dtype, tag=tag, bufs=bufs, name=f"t{b}_{m}")
            if idx % 2 == 0:
                nc.scalar.activation(t[:], gt_ps[:], AF.Relu)
            else:
                nc.vector.tensor_scalar_max(t[:], gt_ps[:], 0.0)
            return t

        def relu_ga(b, m, audT, dtype, pool, tag, bufs):
            """relu(aw * G_a) for (b, m) -> tile in `pool`"""
            idx = b * SC + m
            msl = slice(m * 128, (m + 1) * 128)
            ga_ps = ps_g.tile([128, S], F32, tag="g", name=f"ga{b}_{m}")
            nc.tensor.matmul(ga_ps[:], audT[:, msl], audT[:])
            r_t = pool.tile([128, S], dtype, tag=tag, bufs=bufs,
                            name=f"r{b}_{m}")
            if aw >= 0.0:
                if aw == 1.0 and idx % 2 == 0:
                    nc.vector.tensor_scalar_max(r_t[:], ga_ps[:], 0.0)
                else:
                    nc.scalar.activation(r_t[:], ga_ps[:], AF.Relu,
                                         scale=float(aw))
            else:
                nc.scalar.activation(r_t[:], ga_ps[:], AF.Relu)
                r2 = pool.tile([128, S], dtype, tag=tag, bufs=bufs,
                               name=f"r2{b}_{m}")
                nc.vector.tensor_scalar_mul(r2[:], r_t[:], float(aw))
                r_t = r2
            return r_t

        rb_pool = ctx.enter_context(tc.tile_pool(name="rbank", bufs=1))
        for b in range(N_PRE):
            audT = audio_proc(b)
            for m in range(SC):
                bank[("t", b, m)] = relu_gt_raw(b, m, BF16, rb_pool, "tb",
                                                SC * N_PRE)
                bank[("a", b, m)] = relu_ga(b, m, audT, BF16, rb_pool, "rb",
                                            SC * N_PRE)

        # ---------------- per-batch fusion ----------------
        for b in range(BPC):
            audT = None if b < N_PRE else audio_proc(b)
            am_b = smalls.tile([1, S], F32, tag="amb", bufs=1, name=f"am{b}")
            nc.sync.dma_start(am_b[:], am_d[b:b + 1, :])

            # ---- decoupled row-0 softmax chain (tiny ops only) ----
            # row 0 of each gram via a [1,512] matmul against column 0.
            gtr_ps = ps_mm.tile([1, S], F32, tag="proj", name=f"gtr{b}")
            nc.tensor.matmul(gtr_ps[:], textT_all[:, b, 0:1],
                             textT_all[:, b, :])
            rt_row = smalls.tile([1, S], F32, tag="rtrow", bufs=1,
                                 name=f"rtr{b}")
            nc.scalar.activation(rt_row[:], gtr_ps[:], AF.Relu,
                                 scale=inv_w2_r[0:1, 0:1])
            if b >= N_PRE:
                gar_ps = ps_mm.tile([1, S], F32, tag="proj", name=f"gar{b}")
                nc.tensor.matmul(gar_ps[:], audT[:, 0:1], audT[:])
            ra_row = smalls.tile([1, S], F32, tag="rarow", bufs=1,
                                 name=f"rar{b}")
            if b < N_PRE:
                # row 0 of relu(aw*G_a) == row 0 of the banked m=0 tile
                nc.vector.tensor_copy(ra_row[:], bank[("a", b, 0)][0:1, :])
            else:
                if aw >= 0.0:
                    nc.scalar.activation(ra_row[:], gar_ps[:], AF.Relu,
                                         scale=float(aw))
                else:
                    nc.scalar.activation(ra_row[:], gar_ps[:], AF.Relu)
                    nc.vector.tensor_scalar_mul(ra_row[:], ra_row[:],
                                                float(aw))
            z = smalls.tile([1, S], F32, tag="z", bufs=1, name=f"z{b}")
            if tw == 1.0:
                nc.vector.tensor_add(z[:], rt_row[:], ra_row[:])
            else:
                nc.vector.scalar_tensor_tensor(z[:], rt_row[:], float(tw),
                                               ra_row[:], op0=ALU.mult,
                                               op1=ALU.add)
            nc.vector.tensor_add(z[:], z[:], am_b[:])
            mx = smalls.tile([1, 1], F32, tag="mx")
            nc.vector.reduce_max(mx[:], z[:], axis=AX.X)
            mneg = smalls.tile([1, 1], F32, tag="mneg")
            nc.vector.tensor_scalar_mul(mneg[:], mx[:], -1.0)
            e_row = smalls.tile([1, S], F32, tag="erow", bufs=1, name=f"e{b}")
            sumexp = smalls.tile([1, 1], F32, tag="sumexp")
            nc.scalar.activation(e_row[:], z[:], AF.Exp,
                                 bias=mneg[:], accum_out=sumexp[:])
            rcp = smalls.tile([1, 1], F32, tag="rcp")
            nc.vector.reciprocal(rcp[:], sumexp[:])
            rcp_r = smalls.tile([128, 1], F32, tag="rcpr")
            nc.gpsimd.partition_broadcast(rcp_r[:], rcp[:])
            e_rep = work.tile([128, S], F32, tag="erep", bufs=1, name=f"er{b}")
            nc.gpsimd.partition_broadcast(e_rep[:], e_row[:])

            fdnum = smalls.tile([128, HC], F32, tag="fdnum")
            for c in range(HC):
                scr = work.tile([128, S], F32, tag="ttrscr", bufs=1,
                                name=f"scr{b}_{c}")
                nc.vector.scalar_tensor_tensor(
                    scr[:], hidT_all[:, b, c, :].bitcast(F32),
                    1.0, e_rep[:],
                    op0=ALU.mult, op1=ALU.mult,
                    accum_out=fdnum[:, c:c + 1])
            nc.vector.scalar_tensor_tensor(
                fd_all[:, :, b], fdnum[:], rcp_r[:, 0:1],
                hidT_all[:, b, :, 0].bitcast(F32),
                op0=ALU.mult, op1=ALU.add)

            # ---- bulk [128,512] tile pipeline ----
            for m in range(SC):
                idx = b * SC + m
                msl = slice(m * 128, (m + 1) * 128)
                a_t = att.tile([128, S], F32, tag="att", name=f"a{b}_{m}")
                if ("t", b, m) in bank:
                    nc.scalar.activation(a_t[:], bank[("t", b, m)][:],
                                         AF.Copy, scale=inv_w2_r[:])
                else:
                    gt_ps = ps_g.tile([128, S], F32, tag="g", name=f"gt{b}_{m}")
                    nc.tensor.matmul(gt_ps[:], textT_all[:, b, msl],
                                     textT_all[:, b, :])
                    nc.scalar.activation(a_t[:], gt_ps[:], AF.Relu,
                                         scale=inv_w2_r[:])
                nc.sync.dma_start(tatt_d[b, msl, :], a_t[:])

                if ("a", b, m) in bank:
                    r_t = bank[("a", b, m)]
                else:
                    r_t = relu_ga(b, m, audT, F32, att, "att", 8)

                f_pre = att.tile([128, S], F32, tag="att", name=f"f{b}_{m}")
                if tw == 1.0:
                    nc.vector.tensor_add(f_pre[:], a_t[:], r_t[:])
                else:
                    nc.vector.scalar_tensor_tensor(
                        f_pre[:], a_t[:], float(tw), r_t[:],
                        op0=ALU.mult, op1=ALU.add)

                if fb == 0.0 and tw >= 0.0 and aw >= 0.0:
                    f_out = f_pre
                elif tw >= 0.0 and aw >= 0.0 and fb >= 0.0:
                    f_out = att.tile([128, S], F32, tag="att", name=f"fo{b}_{m}")
                    nc.scalar.add(f_out[:], f_pre[:], float(fb))
                else:
                    f_out = att.tile([128, S], F32, tag="att", name=f"fo{b}_{m}")
                    fb_t = smalls.tile([128, 1], F32, tag="fbt", bufs=1)
                    nc.gpsimd.memset(fb_t[:], float(fb))
                    nc.scalar.activation(f_out[:], f_pre[:], AF.Relu,
                                         bias=fb_t[:])
                nc.sync.dma_start(fatt_d[b, msl, :], f_out[:])

        # ---------------- dense + layernorm on row-0 states ----------------
        h0a = ps_g.tile([BPC, 512], F32, tag="g")
        h0b = ps_g.tile([BPC, H - 512], F32, tag="g")
        for c in range(HC):
            nc.tensor.matmul(h0a[:], fd_all[:, c, :], dwT[:, c, 0:512],
                             start=(c == 0), stop=(c == HC - 1))
            nc.tensor.matmul(h0b[:], fd_all[:, c, :], dwT[:, c, 512:H],
                             start=(c == 0), stop=(c == HC - 1))
        xb = smalls.tile([BPC, H], F32, tag="xln", bufs=2)
        nc.vector.tensor_add(xb[:, 0:512], h0a[:], db_r[:, 0:512])
        nc.vector.tensor_add(xb[:, 512:H], h0b[:], db_r[:, 512:H])
        usum = smalls.tile([BPC, 1], F32, tag="usum")
        nc.vector.reduce_sum(usum[:], xb[:], axis=AX.X)
        uneg = smalls.tile([BPC, 1], F32, tag="uneg")
        nc.vector.tensor_scalar_mul(uneg[:], usum[:], -1.0 / H)
        xc = smalls.tile([BPC, H], F32, tag="xln", bufs=2)
        nc.vector.tensor_scalar_add(xc[:], xb[:], uneg[:])
        sq2 = smalls.tile([BPC, H], F32, tag="xln", bufs=2)
        v = smalls.tile([BPC, 1], F32, tag="v")
        nc.scalar.activation(sq2[:], xc[:], AF.Square, accum_out=v[:])
        eps_t = smalls.tile([BPC, 1], F32, tag="eps", bufs=1)
        nc.gpsimd.memset(eps_t[:], float(LN_EPS))
        std = smalls.tile([BPC, 1], F32, tag="std")
        nc.scalar.activation(std[:], v[:], AF.Sqrt, scale=1.0 / H,
                             bias=eps_t[:])
        rstd = smalls.tile([BPC, 1], F32, tag="rstd")
        nc.vector.reciprocal(rstd[:], std[:])
        y1 = smalls.tile([BPC, H], F32, tag="xln", bufs=2)
        nc.vector.scalar_tensor_tensor(y1[:], xc[:], rstd[:, 0:1], lnw_r[:],
                                       op0=ALU.mult, op1=ALU.mult)
        y2 = smalls.tile([BPC, H], F32, tag="xln", bufs=2)
        nc.vector.tensor_add(y2[:], y1[:], lnb_r[:])
        nc.sync.dma_start(out0_d[:], y2[:])


# ------------------------------------------------------------------
# NTFF profiling hook (only used when PROFILE=True)
# ------------------------------------------------------------------
def _install_profile_hook():
    try:
        import antenv.axon_hooks  # noqa
        return
    except ImportError:
        pass
    so_path = "/opt/axon/libaxon_pjrt.so"
    try:
        lib = ctypes.CDLL(so_path)
    except OSError:
        return
    if not hasattr(lib, "axon_start_nrt_profile"):
        return
    lib.axon_start_nrt_profile.argtypes = [ctypes.POINTER(ctypes.c_int64),
                                           ctypes.c_size_t]
    lib.axon_start_nrt_profile.restype = ctypes.c_int64
    lib.axon_stop_nrt_profile.argtypes = [ctypes.c_char_p]
    lib.axon_stop_nrt_profile.restype = ctypes.c_int64

    @contextlib.contextmanager
    def _hook(output_dir, device_ids):
        import jax
        jax.devices()
        if device_ids:
            ids = (ctypes.c_int64 * len(device_ids))(*device_ids)
            rc = lib.axon_start_nrt_profile(ids, len(device_ids))
        else:
            rc = lib.axon_start_nrt_profile(None, 0)
        if rc != 0:
            raise RuntimeError(f"axon_start_nrt_profile rc={rc}")
        try:
            yield
        finally:
            n = lib.axon_stop_nrt_profile(str(output_dir).encode())
            if n < 0:
                raise RuntimeError(f"axon_stop_nrt_profile rc={n}")

    mod = types.ModuleType("antenv.axon_hooks")
    _hook_box = [_hook]
    mod.get_axon_ntff_profile_hook = lambda: _hook_box[0]
    mod.set_axon_ntff_profile_hook = lambda h: _hook_box.__setitem__(0, h)
    sys.modules["antenv.axon_hooks"] = mod
    import antenv
    antenv.axon_hooks = mod


# ------------------------------------------------------------------
# Host wrapper
# ------------------------------------------------------------------
def kernel(hidden_states, audio_data, attention_mask, Wt, Wa, text_w, audio_w,
           fbias, dense_W, dense_b, ln_w, ln_b):
    global LAST_EXEC_NS
    hs = np.ascontiguousarray(np.asarray(hidden_states, np.float32))
    ad = np.ascontiguousarray(np.asarray(audio_data, np.float32))
    am = np.ascontiguousarray(
        np.asarray(attention_mask, np.float32).reshape(B, S))
    wt = np.ascontiguousarray(np.asarray(Wt, np.float32))
    wa = np.ascontiguousarray(np.asarray(Wa, np.float32))
    dw = np.ascontiguousarray(np.asarray(dense_W, np.float32))
    db = np.ascontiguousarray(np.asarray(dense_b, np.float32))
    lnw = np.ascontiguousarray(np.asarray(ln_w, np.float32))
    lnb = np.ascontiguousarray(np.asarray(ln_b, np.float32))
    tw = float(np.asarray(text_w).reshape(-1)[0])
    aw = float(np.asarray(audio_w).reshape(-1)[0])
    fb = float(np.asarray(fbias).reshape(-1)[0])

    key = (tw, aw, fb)
    if key not in _compiled:
        _compiled[key] = _build(tw, aw, fb)
    nc = _compiled[key]

    in_maps = []
    for i in range(N_CORES):
        sl = slice(i * BPC, (i + 1) * BPC)
        in_maps.append({
            "hid": hs[sl], "aud": ad[sl], "am": am[sl],
            "wt": wt, "wa": wa, "dw": dw, "db": db,
            "lnw": lnw, "lnb": lnb,
        })

    if PROFILE:
        _install_profile_hook()
    res = run_bass_kernel_spmd(nc, in_maps, list(range(N_CORES)),
                               trace=PROFILE)
    LAST_EXEC_NS = res.exec_time_ns

    h0 = np.concatenate([res.results[i]["out0"] for i in range(N_CORES)], 0)
    t_att = np.concatenate([res.results[i]["t_att"] for i in range(N_CORES)], 0)
    f_att = np.concatenate([res.results[i]["f_att"] for i in range(N_CORES)], 0)
    return h0, t_att, f_att
